# revision 2
# baseline (speedup 1.0000x reference)
"""BLOutputLayer forward: out[i] = features[rules[i]] — a rule-book gather.

Strategy (8 NeuronCores, data-parallel over output rows):
  - rules [524288] -> 8 shards of 65536 rows, one per core; features
    [200000, 64] f32 replicated to every core's DRAM.

  v2 (fast path): the int16-indexed SWDGE gather/scatter instructions
  (InstDMAGatherAnt / InstDMAScatterAddAnt) move one 256 B row per
  descriptor with only ~1 us fixed cost per *instruction*, so we want
  few instructions with many indices each. int16 limits reach to 32768
  rows, so the host buckets each core's (feature_idx, out_pos) pairs by
  (feature chunk of 32768 rows x output group of 32768 rows) = 14
  buckets. Per bucket: dma_gather (chunk-local idx) -> SBUF tile in
  [i%128, i//128] layout -> dma_scatter_add (group-local pos) into the
  pre-zeroed DRAM output (add == write). Host work touches only the
  4 MB index array; all 268 MB of data movement stays on device.

  v1 (simple fallback): 512 indirect DMAs of 128 rows (one index per
  partition), ~1.5 us each -> ~758 us/core. Kept for reference.
"""

import numpy as np

import concourse.bacc as bacc
import concourse.mybir as mybir
import concourse.tile as tile
from concourse.bass import IndirectOffsetOnAxis
from concourse.bass_utils import run_bass_kernel_spmd

N_ACTIVE = 200000
C = 64
N_ROWS = 524288
N_CORES = 8
ROWS_PER_CORE = N_ROWS // N_CORES  # 65536
P = 128

CHUNK = 32768  # feature rows addressable by int16 gather indices
N_CHUNKS = (N_ACTIVE + CHUNK - 1) // CHUNK  # 7 (last chunk 3392 rows)
GRP = 32256  # real output rows per scatter group (+256 scratch rows)
SCRATCH = 256
GRPW = GRP + SCRATCH  # 32512 <= 32768, int16-addressable window
N_GRP = (ROWS_PER_CORE + GRP - 1) // GRP  # 3 (last group 1024 real rows)
N_BUCKETS = N_CHUNKS * N_GRP  # 21
OUT_ROWS = N_GRP * GRPW  # device output buffer rows (incl. scratch)

_cache = {}


def _wrap16(a):
    """[S] -> [128, S//16] int16 in the SWDGE wrapped layout: entry j at
    (j % 16, j // 16), replicated across the eight 16-partition groups."""
    w = a.reshape(-1, 16).T  # [16, S//16]
    return np.tile(w, (8, 1)).copy()


def plan_v2(rules_i32):
    """Bucket (idx, pos) pairs per core; pad every bucket to a shared static
    size with SAFE entries (gather: row 0; scatter: scratch rows) so counts
    are compile-time constants."""
    shards = rules_i32.reshape(N_CORES, ROWS_PER_CORE).astype(np.int64)
    pos = np.arange(ROWS_PER_CORE, dtype=np.int64)
    grp_of = pos // GRP
    per_core = []
    counts_all = np.zeros((N_CORES, N_BUCKETS), dtype=np.int64)
    for c in range(N_CORES):
        idx = shards[c]
        key = (idx >> 15) * N_GRP + grp_of
        order = np.argsort(key, kind="stable")
        counts_all[c] = np.bincount(key, minlength=N_BUCKETS)
        per_core.append((idx[order], pos[order]))
    S = np.maximum(((counts_all.max(axis=0) + 127) // 128) * 128, 128).astype(int)
    tot_cols = int(S.sum()) // 16

    gidx_w = np.empty((N_CORES, P, tot_cols), dtype=np.int16)
    sidx_w = np.empty((N_CORES, P, tot_cols), dtype=np.int16)
    for c in range(N_CORES):
        idx_s, pos_s = per_core[c]
        counts = counts_all[c]
        starts = np.concatenate([[0], np.cumsum(counts)])
        col = 0
        for b in range(N_BUCKETS):
            chunk, grp = divmod(b, N_GRP)
            n, s_b = int(counts[b]), int(S[b])
            npad = s_b - n
            g = np.empty(s_b, dtype=np.int16)
            s_ = np.empty(s_b, dtype=np.int16)
            sel = slice(starts[b], starts[b] + n)
            g[:n] = (idx_s[sel] - chunk * CHUNK).astype(np.int16)
            s_[:n] = (pos_s[sel] - grp * GRP).astype(np.int16)
            g[n:] = 0  # safe: reads chunk row 0
            s_[n:] = GRP + (np.arange(npad) % SCRATCH)  # safe: scratch rows
            w = s_b // 16
            gidx_w[c, :, col : col + w] = _wrap16(g)
            sidx_w[c, :, col : col + w] = _wrap16(s_)
            col += w
    return tuple(S.tolist()), gidx_w, sidx_w


def build_v2(S, reps=1, dynamic_reps=False, mode="full"):
    nc = bacc.Bacc("TRN2", target_bir_lowering=False, num_swdge_queues=4)
    tot_cols = sum(S) // 16
    features = nc.dram_tensor(
        "features", [N_ACTIVE, C], mybir.dt.float32, kind="ExternalInput"
    )
    gidx = nc.dram_tensor("gidx", [P, tot_cols], mybir.dt.int16, kind="ExternalInput")
    sidx = nc.dram_tensor("sidx", [P, tot_cols], mybir.dt.int16, kind="ExternalInput")
    if dynamic_reps:
        cnt = nc.dram_tensor("cnt", [1, 16], mybir.dt.int32, kind="ExternalInput")
    out = nc.dram_tensor(
        "out", [OUT_ROWS, C], mybir.dt.float32, kind="ExternalOutput"
    )

    col_of = []
    col = 0
    for b in range(N_BUCKETS):
        col_of.append(col)
        col += S[b] // 16

    # queue assignment: round-robin in emission order keeps adjacent
    # buckets on different queues (pipelining); measured faster than a
    # size-balanced greedy assignment despite ~30% aggregate imbalance
    q_of = [b % 4 for b in range(N_BUCKETS)]

    with tile.TileContext(nc) as tc:
        with (
            tc.tile_pool(name="idx", bufs=1) as idx_pool,
            tc.tile_pool(name="data", bufs=1) as data_pool,
        ):
            gidx_t = idx_pool.tile([P, tot_cols], mybir.dt.int16, tag="gidx")
            sidx_t = idx_pool.tile([P, tot_cols], mybir.dt.int16, tag="sidx")
            nc.sync.dma_start(out=gidx_t[:], in_=gidx[:])
            nc.sync.dma_start(out=sidx_t[:], in_=sidx[:])
            if dynamic_reps:
                cnt_t = idx_pool.tile([1, 16], mybir.dt.int32, tag="cnt")
                nc.sync.dma_start(out=cnt_t[:], in_=cnt[:])

            def body():
                tiles = []
                for b in range(N_BUCKETS):
                    chunk, _grp = divmod(b, N_GRP)
                    s_b = S[b]
                    w = s_b // 16
                    c_end = min((chunk + 1) * CHUNK, N_ACTIVE)
                    data_t = data_pool.tile(
                        [P, s_b // 128, C], mybir.dt.float32, tag=f"data{b}"
                    )
                    tiles.append(data_t)
                    if mode != "scatter":
                        nc.gpsimd.dma_gather(
                            data_t[:],
                            features[chunk * CHUNK : c_end],
                            gidx_t[:, col_of[b] : col_of[b] + w],
                            num_idxs=s_b,
                            num_idxs_reg=s_b,
                            elem_size=C,
                            elem_step=C,
                            single_packet=False,
                            queue_num=q_of[b],
                        )
                    else:
                        nc.vector.memset(data_t[:], 0)
                for b in range(N_BUCKETS):
                    _chunk, grp = divmod(b, N_GRP)
                    s_b = S[b]
                    w = s_b // 16
                    if mode == "gather":
                        nc.sync.dma_start(
                            out=out[b * 128 : b * 128 + 128].rearrange(
                                "(p n) c -> p (n c)", p=P
                            ),
                            in_=tiles[b][:, :1, :].rearrange("p n c -> p (n c)"),
                        )
                    else:
                        nc.gpsimd.dma_scatter_add(
                            out[grp * GRPW : (grp + 1) * GRPW],
                            tiles[b][:],
                            sidx_t[:, col_of[b] : col_of[b] + w],
                            num_idxs=s_b,
                            num_idxs_reg=s_b,
                            elem_size=C,
                            elem_step=C,
                            single_packet=False,
                            queue_num=q_of[b],
                        )

            if dynamic_reps:
                rregs = nc.alloc_registers("reps")
                nc.regs_load(rregs, cnt_t[:1, 15:16])
                reps_val = nc.snap(rregs, donate=True)
                with tc.For_i(0, reps_val) as _i:
                    body()
            else:
                for _rep in range(reps):
                    body()
    nc.finalize()
    return nc


def run(features, rules, reps=1):
    features = np.ascontiguousarray(np.asarray(features), dtype=np.float32)
    rules_i32 = np.ascontiguousarray(np.asarray(rules)).astype(np.int32)

    S, gidx_w, sidx_w = plan_v2(rules_i32)
    key = ("v3", S, reps)
    if _cache.get("key") != key:
        _cache["nc"] = build_v2(S, reps)
        _cache["key"] = key
    nc = _cache["nc"]

    in_maps = [
        {"features": features, "gidx": gidx_w[c], "sidx": sidx_w[c]}
        for c in range(N_CORES)
    ]
    res = run_bass_kernel_spmd(nc, in_maps, list(range(N_CORES)))
    outs = []
    for c in range(N_CORES):
        buf = res.results[c]["out"].reshape(N_GRP, GRPW, C)
        outs.append(buf[:, :GRP].reshape(-1, C)[:ROWS_PER_CORE])
    full = np.concatenate(outs, axis=0)
    return full, res


def kernel(**inputs):
    full, _ = run(inputs["features"], inputs["rules"])
    return full


def measure_hw_ns(features, rules, r_lo=4, r_hi=64):
    """Rep-slope HW exec time (ns) on a dynamic-reps build. Test-only helper;
    imports bench lazily so kernel.py stays self-contained for the harness."""
    from bench import BassRunner

    features = np.ascontiguousarray(np.asarray(features), dtype=np.float32)
    rules_i32 = np.ascontiguousarray(np.asarray(rules)).astype(np.int32)
    S, gidx_w, sidx_w = plan_v2(rules_i32)
    nc = build_v2(S, dynamic_reps=True)

    def with_reps(r):
        return [
            {
                "features": features,
                "gidx": gidx_w[c],
                "sidx": sidx_w[c],
                "cnt": np.array([[0] * 15 + [r]], np.int32),
            }
            for c in range(N_CORES)
        ]

    runner = BassRunner(nc, with_reps(r_lo))
    return runner.time_reps(with_reps, r_lo, r_hi, verbose=True)


# ---------------------------------------------------------------------------
# v1 (simple indirect-DMA version, ~758 us/core) kept for reference/benching
N_GATHERS = ROWS_PER_CORE // P  # 512
G = 32
N_GROUPS = N_GATHERS // G  # 16


def _build(reps=1):
    nc = bacc.Bacc("TRN2", target_bir_lowering=False)
    features = nc.dram_tensor(
        "features", [N_ACTIVE, C], mybir.dt.float32, kind="ExternalInput"
    )
    rules = nc.dram_tensor(
        "rules", [P, N_GATHERS], mybir.dt.int32, kind="ExternalInput"
    )
    out = nc.dram_tensor(
        "out", [N_GROUPS, P, G, C], mybir.dt.float32, kind="ExternalOutput"
    )

    with tile.TileContext(nc) as tc:
        with (
            tc.tile_pool(name="idx", bufs=1) as idx_pool,
            tc.tile_pool(name="data", bufs=1) as data_pool,
        ):
            idx_tile = idx_pool.tile([P, N_GATHERS], mybir.dt.int32, tag="idx")
            nc.sync.dma_start(out=idx_tile[:], in_=rules[:])
            for _rep in range(reps):
                for grp in range(N_GROUPS):
                    data_tile = data_pool.tile([P, G, C], mybir.dt.float32, tag="data")
                    for g in range(G):
                        j = grp * G + g
                        nc.gpsimd.indirect_dma_start(
                            out=data_tile[:, g],
                            out_offset=None,
                            in_=features[:],
                            in_offset=IndirectOffsetOnAxis(
                                ap=idx_tile[:, j : j + 1], axis=0
                            ),
                        )
                    nc.sync.dma_start(out=out[grp], in_=data_tile[:])
    nc.finalize()
    return nc



# revision 3
# speedup vs baseline: 1.5361x; 1.5361x over previous
"""BLOutputLayer forward: out[i] = features[rules[i]] — a rule-book gather.

Strategy (8 NeuronCores, data-parallel over output rows):
  - rules [524288] -> 8 shards of 65536 rows, one per core; features
    [200000, 64] f32 replicated to every core's DRAM.

  v2 (fast path): the int16-indexed SWDGE gather/scatter instructions
  (InstDMAGatherAnt / InstDMAScatterAddAnt) move one 256 B row per
  descriptor with only ~1 us fixed cost per *instruction*, so we want
  few instructions with many indices each. int16 limits reach to 32768
  rows, so the host buckets each core's (feature_idx, out_pos) pairs by
  (feature chunk of 32768 rows x output group of 32768 rows) = 14
  buckets. Per bucket: dma_gather (chunk-local idx) -> SBUF tile in
  [i%128, i//128] layout -> dma_scatter_add (group-local pos) into the
  pre-zeroed DRAM output (add == write). Host work touches only the
  4 MB index array; all 268 MB of data movement stays on device.

  v1 (simple fallback): 512 indirect DMAs of 128 rows (one index per
  partition), ~1.5 us each -> ~758 us/core. Kept for reference.
"""

import numpy as np

import concourse.bacc as bacc
import concourse.mybir as mybir
import concourse.tile as tile
from concourse.bass import IndirectOffsetOnAxis
from concourse.bass_utils import run_bass_kernel_spmd

N_ACTIVE = 200000
C = 64
N_ROWS = 524288
N_CORES = 8
ROWS_PER_CORE = N_ROWS // N_CORES  # 65536
P = 128

CHUNK = 32768  # feature rows addressable by int16 gather indices
N_CHUNKS = (N_ACTIVE + CHUNK - 1) // CHUNK  # 7 (last chunk 3392 rows)
GRP = 32256  # real output rows per scatter group (+256 scratch rows)
SCRATCH = 256
GRPW = GRP + SCRATCH  # 32512 <= 32768, int16-addressable window
N_GRP = (ROWS_PER_CORE + GRP - 1) // GRP  # 3 (last group 1024 real rows)
N_BUCKETS = N_CHUNKS * N_GRP  # 21
OUT_ROWS = N_GRP * GRPW  # device output buffer rows (incl. scratch)

_cache = {}


def _wrap16(a):
    """[S] -> [128, S//16] int16 in the SWDGE wrapped layout: entry j at
    (j % 16, j // 16), replicated across the eight 16-partition groups."""
    w = a.reshape(-1, 16).T  # [16, S//16]
    return np.tile(w, (8, 1)).copy()


def plan_v2(rules_i32):
    """Bucket (idx, pos) pairs per core; pad every bucket to a shared static
    size with SAFE entries (gather: row 0; scatter: scratch rows) so counts
    are compile-time constants."""
    shards = rules_i32.reshape(N_CORES, ROWS_PER_CORE).astype(np.int64)
    pos = np.arange(ROWS_PER_CORE, dtype=np.int64)
    grp_of = pos // GRP
    per_core = []
    counts_all = np.zeros((N_CORES, N_BUCKETS), dtype=np.int64)
    for c in range(N_CORES):
        idx = shards[c]
        key = (idx >> 15) * N_GRP + grp_of
        order = np.argsort(key, kind="stable")
        counts_all[c] = np.bincount(key, minlength=N_BUCKETS)
        per_core.append((idx[order], pos[order]))
    S = np.maximum(((counts_all.max(axis=0) + 127) // 128) * 128, 128).astype(int)
    tot_cols = int(S.sum()) // 16

    gidx_w = np.empty((N_CORES, P, tot_cols), dtype=np.int16)
    sidx_w = np.empty((N_CORES, P, tot_cols), dtype=np.int16)
    for c in range(N_CORES):
        idx_s, pos_s = per_core[c]
        counts = counts_all[c]
        starts = np.concatenate([[0], np.cumsum(counts)])
        col = 0
        for b in range(N_BUCKETS):
            chunk, grp = divmod(b, N_GRP)
            n, s_b = int(counts[b]), int(S[b])
            npad = s_b - n
            g = np.empty(s_b, dtype=np.int16)
            s_ = np.empty(s_b, dtype=np.int16)
            sel = slice(starts[b], starts[b] + n)
            g[:n] = (idx_s[sel] - chunk * CHUNK).astype(np.int16)
            s_[:n] = (pos_s[sel] - grp * GRP).astype(np.int16)
            g[n:] = 0  # safe: reads chunk row 0
            s_[n:] = GRP + (np.arange(npad) % SCRATCH)  # safe: scratch rows
            w = s_b // 16
            gidx_w[c, :, col : col + w] = _wrap16(g)
            sidx_w[c, :, col : col + w] = _wrap16(s_)
            col += w
    return tuple(S.tolist()), gidx_w, sidx_w


def build_v2(S, reps=1, dynamic_reps=False, mode="full"):
    nc = bacc.Bacc("TRN2", target_bir_lowering=False, num_swdge_queues=4)
    tot_cols = sum(S) // 16
    features = nc.dram_tensor(
        "features", [N_ACTIVE, C], mybir.dt.float32, kind="ExternalInput"
    )
    gidx = nc.dram_tensor("gidx", [P, tot_cols], mybir.dt.int16, kind="ExternalInput")
    sidx = nc.dram_tensor("sidx", [P, tot_cols], mybir.dt.int16, kind="ExternalInput")
    if dynamic_reps:
        cnt = nc.dram_tensor("cnt", [1, 16], mybir.dt.int32, kind="ExternalInput")
    out = nc.dram_tensor(
        "out", [OUT_ROWS, C], mybir.dt.float32, kind="ExternalOutput"
    )

    col_of = []
    col = 0
    for b in range(N_BUCKETS):
        col_of.append(col)
        col += S[b] // 16

    # queue assignment: round-robin in emission order keeps adjacent
    # buckets on different queues (pipelining); measured faster than a
    # size-balanced greedy assignment despite ~30% aggregate imbalance
    q_of = [b % 4 for b in range(N_BUCKETS)]

    with tile.TileContext(nc) as tc:
        with (
            tc.tile_pool(name="idx", bufs=1) as idx_pool,
            tc.tile_pool(name="data", bufs=1) as data_pool,
        ):
            gidx_t = idx_pool.tile([P, tot_cols], mybir.dt.int16, tag="gidx")
            sidx_t = idx_pool.tile([P, tot_cols], mybir.dt.int16, tag="sidx")
            nc.sync.dma_start(out=gidx_t[:], in_=gidx[:])
            nc.sync.dma_start(out=sidx_t[:], in_=sidx[:])
            if dynamic_reps:
                cnt_t = idx_pool.tile([1, 16], mybir.dt.int32, tag="cnt")
                nc.sync.dma_start(out=cnt_t[:], in_=cnt[:])

            def body():
                tiles = []
                for b in range(N_BUCKETS):
                    chunk, _grp = divmod(b, N_GRP)
                    s_b = S[b]
                    w = s_b // 16
                    c_end = min((chunk + 1) * CHUNK, N_ACTIVE)
                    data_t = data_pool.tile(
                        [P, s_b // 128, C], mybir.dt.float32, tag=f"data{b}"
                    )
                    tiles.append(data_t)
                    if mode != "scatter":
                        nc.gpsimd.dma_gather(
                            data_t[:],
                            features[chunk * CHUNK : c_end],
                            gidx_t[:, col_of[b] : col_of[b] + w],
                            num_idxs=s_b,
                            num_idxs_reg=s_b,
                            elem_size=C,
                            elem_step=C,
                            single_packet=False,
                            queue_num=q_of[b],
                        )
                    else:
                        nc.vector.memset(data_t[:], 0)
                for b in range(N_BUCKETS):
                    _chunk, grp = divmod(b, N_GRP)
                    s_b = S[b]
                    w = s_b // 16
                    if mode == "gather":
                        nc.sync.dma_start(
                            out=out[b * 128 : b * 128 + 128].rearrange(
                                "(p n) c -> p (n c)", p=P
                            ),
                            in_=tiles[b][:, :1, :].rearrange("p n c -> p (n c)"),
                        )
                    else:
                        nc.gpsimd.dma_scatter_add(
                            out[grp * GRPW : (grp + 1) * GRPW],
                            tiles[b][:],
                            sidx_t[:, col_of[b] : col_of[b] + w],
                            num_idxs=s_b,
                            num_idxs_reg=s_b,
                            elem_size=C,
                            elem_step=C,
                            single_packet=False,
                            queue_num=q_of[b],
                        )

            if dynamic_reps:
                rregs = nc.alloc_registers("reps")
                nc.regs_load(rregs, cnt_t[:1, 15:16])
                reps_val = nc.snap(rregs, donate=True)
                with tc.For_i(0, reps_val) as _i:
                    body()
            else:
                for _rep in range(reps):
                    body()
    nc.finalize()
    return nc


def run(features, rules, reps=1):
    features = np.ascontiguousarray(np.asarray(features), dtype=np.float32)
    rules_i32 = np.ascontiguousarray(np.asarray(rules)).astype(np.int32)

    S, gidx_w, sidx_w = plan_v2(rules_i32)
    key = ("v3", S, reps)
    if _cache.get("key") != key:
        _cache["nc"] = build_v2(S, reps)
        _cache["key"] = key
    nc = _cache["nc"]

    in_maps = [
        {"features": features, "gidx": gidx_w[c], "sidx": sidx_w[c]}
        for c in range(N_CORES)
    ]
    res = run_bass_kernel_spmd(nc, in_maps, list(range(N_CORES)))
    outs = []
    for c in range(N_CORES):
        buf = res.results[c]["out"].reshape(N_GRP, GRPW, C)
        outs.append(buf[:, :GRP].reshape(-1, C)[:ROWS_PER_CORE])
    full = np.concatenate(outs, axis=0)
    return full, res


def kernel(**inputs):
    full, _ = run(inputs["features"], inputs["rules"])
    return full


def measure_hw_ns(features, rules, r_lo=64, r_hi=1088):
    """Rep-slope HW exec time (ns) on a dynamic-reps build. Test-only helper;
    imports bench lazily so kernel.py stays self-contained for the harness."""
    from bench import BassRunner

    features = np.ascontiguousarray(np.asarray(features), dtype=np.float32)
    rules_i32 = np.ascontiguousarray(np.asarray(rules)).astype(np.int32)
    S, gidx_w, sidx_w = plan_v2(rules_i32)
    nc = build_v2(S, dynamic_reps=True)

    def with_reps(r):
        return [
            {
                "features": features,
                "gidx": gidx_w[c],
                "sidx": sidx_w[c],
                "cnt": np.array([[0] * 15 + [r]], np.int32),
            }
            for c in range(N_CORES)
        ]

    runner = BassRunner(nc, with_reps(r_lo))
    return runner.time_reps(with_reps, r_lo, r_hi, verbose=True)


# ---------------------------------------------------------------------------
# v1 (simple indirect-DMA version, ~758 us/core) kept for reference/benching
N_GATHERS = ROWS_PER_CORE // P  # 512
G = 32
N_GROUPS = N_GATHERS // G  # 16


def _build(reps=1):
    nc = bacc.Bacc("TRN2", target_bir_lowering=False)
    features = nc.dram_tensor(
        "features", [N_ACTIVE, C], mybir.dt.float32, kind="ExternalInput"
    )
    rules = nc.dram_tensor(
        "rules", [P, N_GATHERS], mybir.dt.int32, kind="ExternalInput"
    )
    out = nc.dram_tensor(
        "out", [N_GROUPS, P, G, C], mybir.dt.float32, kind="ExternalOutput"
    )

    with tile.TileContext(nc) as tc:
        with (
            tc.tile_pool(name="idx", bufs=1) as idx_pool,
            tc.tile_pool(name="data", bufs=1) as data_pool,
        ):
            idx_tile = idx_pool.tile([P, N_GATHERS], mybir.dt.int32, tag="idx")
            nc.sync.dma_start(out=idx_tile[:], in_=rules[:])
            for _rep in range(reps):
                for grp in range(N_GROUPS):
                    data_tile = data_pool.tile([P, G, C], mybir.dt.float32, tag="data")
                    for g in range(G):
                        j = grp * G + g
                        nc.gpsimd.indirect_dma_start(
                            out=data_tile[:, g],
                            out_offset=None,
                            in_=features[:],
                            in_offset=IndirectOffsetOnAxis(
                                ap=idx_tile[:, j : j + 1], axis=0
                            ),
                        )
                    nc.sync.dma_start(out=out[grp], in_=data_tile[:])
    nc.finalize()
    return nc



# revision 5
# speedup vs baseline: 1.6881x; 1.0989x over previous
"""v3.1: two-phase gather with phase-1 dedup + unaligned pair-merge.

Phase 1 gathers each group's UNIQUE table rows once; runs of consecutive
table rows are covered by 512B descriptors (elem_size=128 f32, elem_step=64
-> one descriptor fetches rows r and r+1 via an overlapping strided AP).
Phase 2 gathers staging positions in output order (8 sub-gathers/group) and
writes `out` with big contiguous HWDGE descriptors.
"""

import numpy as np

import concourse.bacc as bacc
import concourse.mybir as mybir
import concourse.tile as tile
from concourse.ap import AP

N_ACTIVE = 200000
C = 64
N_ROWS = 524288
N_CORES = 8
ROWS_PER_CORE = N_ROWS // N_CORES  # 65536
P = 128

CHUNK = 32768
N_CHUNKS = (N_ACTIVE + CHUNK - 1) // CHUNK  # 7
GRP_CAP = 32768  # int16 staging reach per group
N_SUB = 8

_cache = {}


def _wrap16(a):
    w = a.reshape(-1, 16).T
    return np.tile(w, (8, 1)).copy()


def _roundup(x, m):
    return -(-x // m) * m


def _pair_structure(ur):
    """Greedy pairing of sorted unique rows into pairs/singles.

    Returns (first_of_pair_mask, second_of_pair_mask)."""
    m = len(ur)
    if m == 0:
        return np.zeros(0, bool), np.zeros(0, bool)
    new_run = np.ones(m, bool)
    new_run[1:] = np.diff(ur) != 1
    run_id = np.cumsum(new_run) - 1
    run_start = np.flatnonzero(new_run)
    run_len = np.diff(np.append(run_start, m))
    pos = np.arange(m) - run_start[run_id]
    first = (pos % 2 == 0) & (pos + 1 < run_len[run_id])
    second = np.zeros(m, bool)
    second[1:] = first[:-1]
    return first, second


def _plan_group(shards, a, b):
    """Per-core pair/single bucket structure for rows [a:b)."""
    per_core = []
    for c in range(N_CORES):
        idx = shards[c, a:b]
        ur, inv = np.unique(idx, return_inverse=True)
        first, second = _pair_structure(ur)
        chunk_ur = (ur >> 15).astype(np.int64)
        is_single = ~(first | second)
        npairs = np.bincount(chunk_ur[first], minlength=N_CHUNKS)
        nsing = np.bincount(chunk_ur[is_single], minlength=N_CHUNKS)
        per_core.append((ur, inv, first, second, is_single, chunk_ur, npairs, nsing))
    npairs_max = np.max([pc[6] for pc in per_core], axis=0)
    nsing_max = np.max([pc[7] for pc in per_core], axis=0)
    SP = np.where(npairs_max > 0, np.maximum(_roundup(npairs_max, 128), 128), 0)
    SS = np.where(nsing_max > 0, np.maximum(_roundup(nsing_max, 128), 128), 0)
    stg_rows = int((2 * SP + SS).sum())
    return per_core, SP.astype(int), SS.astype(int), stg_rows


def plan_v31(rules_i32):
    shards = rules_i32.reshape(N_CORES, ROWS_PER_CORE).astype(np.int64)

    # greedy group packing (128-row steps): staging rows <= GRP_CAP
    bounds = [0]
    plans = []
    while bounds[-1] < ROWS_PER_CORE:
        a = bounds[-1]
        b = min(a + 34048, ROWS_PER_CORE)
        while True:
            per_core, SP, SS, stg_rows = _plan_group(shards, a, b)
            if stg_rows <= GRP_CAP or b - a <= 128:
                break
            over = stg_rows - GRP_CAP
            b = a + max(128, (b - a) - _roundup(over, 128))
        assert stg_rows <= GRP_CAP, (a, b, stg_rows)
        bounds.append(b)
        plans.append((a, b, per_core, SP, SS, stg_rows))

    groups_static = []
    g1_parts = [[] for _ in range(N_CORES)]
    g2_parts = [[] for _ in range(N_CORES)]
    for a, b, per_core, SP, SS, stg_rows in plans:
        n = b - a
        assert n % 128 == 0
        slot_base = []
        o = 0
        for k in range(N_CHUNKS):
            slot_base.append((o, o + 2 * SP[k]))  # (pairs base, singles base)
            o += 2 * SP[k] + SS[k]
        Ct = n // 128
        base_cols = Ct // N_SUB
        rem = Ct % N_SUB
        subs = []
        oc = 0
        for s in range(N_SUB):
            cs = base_cols + (1 if s < rem else 0)
            if cs > 0:
                subs.append((oc, cs))
            oc += cs
        groups_static.append(
            dict(a=a, b=b, SP=tuple(SP), SS=tuple(SS), subs=tuple(subs))
        )

        for c in range(N_CORES):
            ur, inv, first, second, is_single, chunk_ur, npairs, nsing = per_core[c]
            pos_ur = np.empty(len(ur), dtype=np.int64)
            for k in range(N_CHUNKS):
                pb, sb = slot_base[k]
                cols_p = SP[k] // 128
                cols_s = SS[k] // 128
                sel = first & (chunk_ur == k)
                rows = ur[sel]
                np_k = len(rows)
                j = np.arange(np_k)
                p1 = pb + (j % 128) * (2 * cols_p) + 2 * (j // 128)
                pos_ur[sel] = p1
                sel2 = np.zeros(len(ur), bool)
                sel2[1:] = sel[:-1]
                pos_ur[sel2] = p1 + 1
                if SP[k] > 0:
                    gi = np.zeros(SP[k], dtype=np.int16)
                    gi[:np_k] = (rows - k * CHUNK).astype(np.int16)
                    g1_parts[c].append(_wrap16(gi))
                sel = is_single & (chunk_ur == k)
                rows = ur[sel]
                ns_k = len(rows)
                j = np.arange(ns_k)
                pos_ur[sel] = sb + (j % 128) * cols_s + j // 128
                if SS[k] > 0:
                    gi = np.zeros(SS[k], dtype=np.int16)
                    gi[:ns_k] = (rows - k * CHUNK).astype(np.int16)
                    g1_parts[c].append(_wrap16(gi))
            stagpos = pos_ur[inv]
            assert stagpos.max() < GRP_CAP
            for o_s, C_s in subs:
                block = stagpos[128 * o_s : 128 * (o_s + C_s)]
                PM = block.reshape(128, C_s)
                stream = PM.T.ravel().astype(np.int16)
                g2_parts[c].append(_wrap16(stream))

    g1idx_w = np.stack([np.concatenate(p, axis=1) for p in g1_parts])
    g2idx_w = np.stack([np.concatenate(p, axis=1) for p in g2_parts])
    static = tuple(
        (g["a"], g["b"], g["SP"], g["SS"], g["subs"]) for g in groups_static
    )
    return static, g1idx_w, g2idx_w


def build_v31(static, dynamic_reps=False, reps=1, mode="full"):
    nc = bacc.Bacc("TRN2", target_bir_lowering=False, num_swdge_queues=4)
    tot1 = sum(sum(sp) + sum(ss) for (_a, _b, sp, ss, _subs) in static)
    tot2 = ROWS_PER_CORE
    features = nc.dram_tensor(
        "features", [N_ACTIVE, C], mybir.dt.float32, kind="ExternalInput"
    )
    g1idx = nc.dram_tensor(
        "g1idx", [P, tot1 // 16], mybir.dt.int16, kind="ExternalInput"
    )
    g2idx = nc.dram_tensor(
        "g2idx", [P, tot2 // 16], mybir.dt.int16, kind="ExternalInput"
    )
    if dynamic_reps:
        cnt = nc.dram_tensor("cnt", [1, 16], mybir.dt.int32, kind="ExternalInput")
    out = nc.dram_tensor(
        "out", [ROWS_PER_CORE, C], mybir.dt.float32, kind="ExternalOutput"
    )

    qn = [0]
    qload = [0, 0, 0, 0]

    def next_q(descs=0):
        q = min(range(4), key=lambda i: qload[i])
        qload[q] += descs
        return q

    with tile.TileContext(nc) as tc:
        with (
            tc.tile_pool(name="idx", bufs=1) as idx_pool,
            tc.tile_pool(name="p1", bufs=1) as p1_pool,
            tc.tile_pool(name="p2", bufs=1) as p2_pool,
            tc.tile_pool(name="stg", bufs=1, space="DRAM") as stg_pool,
        ):
            g1idx_t = idx_pool.tile([P, tot1 // 16], mybir.dt.int16, tag="g1")
            g2idx_t = idx_pool.tile([P, tot2 // 16], mybir.dt.int16, tag="g2")
            nc.sync.dma_start(out=g1idx_t[:], in_=g1idx[:])
            nc.sync.dma_start(out=g2idx_t[:], in_=g2idx[:])
            if dynamic_reps:
                cnt_t = idx_pool.tile([1, 16], mybir.dt.int32, tag="cnt")
                nc.sync.dma_start(out=cnt_t[:], in_=cnt[:])

            def body():
                col1 = [0]
                col2 = [0]
                for gi_, (a, b, SP, SS, subs) in enumerate(static):
                    stg_rows = sum(2 * s for s in SP) + sum(SS)
                    stg_t = stg_pool.tile(
                        [stg_rows, C], mybir.dt.float32, tag=f"stg{gi_}"
                    )
                    slot = 0
                    for k in range(N_CHUNKS):
                        w_base = k * CHUNK
                        n_window = min(CHUNK, N_ACTIVE - w_base - 1)
                        s_p = SP[k]
                        if s_p > 0:
                            if mode != "p2":
                                cols_p = s_p // 128
                                w = s_p // 16
                                base_ap = features[w_base : w_base + 2]
                                pair_ap = AP(
                                    base_ap.tensor,
                                    base_ap.offset,
                                    [[C, n_window], [1, 2 * C]],
                                )
                                t = p1_pool.tile(
                                    [P, cols_p, 2 * C],
                                    mybir.dt.float32,
                                    tag=f"p1p_{k}",
                                )
                                nc.gpsimd.dma_gather(
                                    t[:],
                                    pair_ap,
                                    g1idx_t[:, col1[0] : col1[0] + w],
                                    num_idxs=s_p,
                                    num_idxs_reg=s_p,
                                    elem_size=2 * C,
                                    elem_step=C,
                                    single_packet=False,
                                    queue_num=next_q(s_p),
                                )
                                nc.sync.dma_start(
                                    out=stg_t[slot : slot + 2 * s_p].rearrange(
                                        "(p n) c -> p (n c)", p=P
                                    ),
                                    in_=t[:].rearrange("p n c -> p (n c)"),
                                )
                            col1[0] += s_p // 16
                            slot += 2 * s_p
                        s_s = SS[k]
                        if s_s > 0:
                            if mode != "p2":
                                cols_s = s_s // 128
                                w = s_s // 16
                                c_end = min(w_base + CHUNK, N_ACTIVE)
                                t = p1_pool.tile(
                                    [P, cols_s, C], mybir.dt.float32, tag=f"p1s_{k}"
                                )
                                nc.gpsimd.dma_gather(
                                    t[:],
                                    features[w_base:c_end],
                                    g1idx_t[:, col1[0] : col1[0] + w],
                                    num_idxs=s_s,
                                    num_idxs_reg=s_s,
                                    elem_size=C,
                                    elem_step=C,
                                    single_packet=False,
                                    queue_num=next_q(s_s),
                                )
                                nc.sync.dma_start(
                                    out=stg_t[slot : slot + s_s].rearrange(
                                        "(p n) c -> p (n c)", p=P
                                    ),
                                    in_=t[:].rearrange("p n c -> p (n c)"),
                                )
                            col1[0] += s_s // 16
                            slot += s_s
                    if mode == "p1":
                        col2[0] += (b - a) // 16
                        continue
                    for si_, (o_s, C_s) in enumerate(subs):
                        n_i = 128 * C_s
                        w = n_i // 16
                        t2 = p2_pool.tile(
                            [P, C_s, C], mybir.dt.float32, tag=f"p2_{si_}"
                        )
                        nc.gpsimd.dma_gather(
                            t2[:],
                            stg_t[:],
                            g2idx_t[:, col2[0] : col2[0] + w],
                            num_idxs=n_i,
                            num_idxs_reg=n_i,
                            elem_size=C,
                            elem_step=C,
                            single_packet=False,
                            queue_num=next_q(n_i),
                        )
                        col2[0] += w
                        A_s = a + 128 * o_s
                        nc.sync.dma_start(
                            out=out[A_s : A_s + n_i].rearrange(
                                "(p n) c -> p (n c)", p=P
                            ),
                            in_=t2[:].rearrange("p n c -> p (n c)"),
                        )

            if dynamic_reps:
                rregs = nc.alloc_registers("reps")
                nc.regs_load(rregs, cnt_t[:1, 15:16])
                reps_val = nc.snap(rregs, donate=True)
                with tc.For_i(0, reps_val) as _i:
                    body()
            else:
                for _ in range(reps):
                    body()
    nc.finalize()
    return nc


def run(features, rules):
    from concourse.bass_utils import run_bass_kernel_spmd

    features = np.ascontiguousarray(np.asarray(features), dtype=np.float32)
    rules_i32 = np.ascontiguousarray(np.asarray(rules)).astype(np.int32)

    static, g1idx_w, g2idx_w = plan_v31(rules_i32)
    key = ("v31-ship-g", static)
    if _cache.get("key") != key:
        _cache["nc"] = build_v31(static)
        _cache["key"] = key
    nc = _cache["nc"]

    in_maps = [
        {"features": features, "g1idx": g1idx_w[c], "g2idx": g2idx_w[c]}
        for c in range(N_CORES)
    ]
    res = run_bass_kernel_spmd(nc, in_maps, list(range(N_CORES)))
    full = np.concatenate([res.results[c]["out"] for c in range(N_CORES)], axis=0)
    return full, res


def kernel(**inputs):
    full, _ = run(inputs["features"], inputs["rules"])
    return full


def measure_hw_ns(features, rules, r_lo=64, r_hi=1088, mode="full"):
    from bench import BassRunner

    features = np.ascontiguousarray(np.asarray(features), dtype=np.float32)
    rules_i32 = np.ascontiguousarray(np.asarray(rules)).astype(np.int32)
    static, g1idx_w, g2idx_w = plan_v31(rules_i32)
    nc = build_v31(static, dynamic_reps=True, mode=mode)

    def with_reps(r):
        return [
            {
                "features": features,
                "g1idx": g1idx_w[c],
                "g2idx": g2idx_w[c],
                "cnt": np.array([[0] * 15 + [r]], np.int32),
            }
            for c in range(N_CORES)
        ]

    runner = BassRunner(nc, with_reps(r_lo))
    return runner.time_reps(with_reps, r_lo, r_hi, verbose=True)


# revision 6
# speedup vs baseline: 1.8605x; 1.1021x over previous
"""v3.2: two-phase gather, dedup + pair-merge, phase-ordered emission.

Phase 1 gathers each group's UNIQUE table rows (shuffled stream order);
consecutive-row runs use 512B descriptors (elem_size=128, elem_step=64 via an
overlapping strided AP). Staging writes and final output writes ride HWDGE
with large contiguous descriptors. Phase 2 gathers staging positions in
output order. ALL phase-1 groups are emitted before ANY phase-2 group so the
4 SWDGE rings never stall on a staging barrier; instructions are assigned to
rings by greedy descriptor-count balancing.

Phase 1 gathers each group's UNIQUE table rows once; runs of consecutive
table rows are covered by 512B descriptors (elem_size=128 f32, elem_step=64
-> one descriptor fetches rows r and r+1 via an overlapping strided AP).
Phase 2 gathers staging positions in output order (8 sub-gathers/group) and
writes `out` with big contiguous HWDGE descriptors.
"""

import numpy as np

import concourse.bacc as bacc
import concourse.mybir as mybir
import concourse.tile as tile
from concourse.ap import AP

N_ACTIVE = 200000
C = 64
N_ROWS = 524288
N_CORES = 8
ROWS_PER_CORE = N_ROWS // N_CORES  # 65536
P = 128

CHUNK = 32768
N_CHUNKS = (N_ACTIVE + CHUNK - 1) // CHUNK  # 7
GRP_CAP = 32768  # int16 staging reach per group
N_SUB = 8

_cache = {}


def _wrap16(a):
    w = a.reshape(-1, 16).T
    return np.tile(w, (8, 1)).copy()


def _roundup(x, m):
    return -(-x // m) * m


def _pair_structure(ur):
    """Greedy pairing of sorted unique rows into pairs/singles.

    Returns (first_of_pair_mask, second_of_pair_mask)."""
    m = len(ur)
    if m == 0:
        return np.zeros(0, bool), np.zeros(0, bool)
    new_run = np.ones(m, bool)
    new_run[1:] = np.diff(ur) != 1
    run_id = np.cumsum(new_run) - 1
    run_start = np.flatnonzero(new_run)
    run_len = np.diff(np.append(run_start, m))
    pos = np.arange(m) - run_start[run_id]
    first = (pos % 2 == 0) & (pos + 1 < run_len[run_id])
    second = np.zeros(m, bool)
    second[1:] = first[:-1]
    return first, second


def _plan_group(shards, a, b, pairing=True):
    """Per-core pair/single bucket structure for rows [a:b)."""
    per_core = []
    for c in range(N_CORES):
        idx = shards[c, a:b]
        ur, inv = np.unique(idx, return_inverse=True)
        if pairing:
            first, second = _pair_structure(ur)
        else:
            first = np.zeros(len(ur), bool)
            second = np.zeros(len(ur), bool)
        chunk_ur = (ur >> 15).astype(np.int64)
        is_single = ~(first | second)
        npairs = np.bincount(chunk_ur[first], minlength=N_CHUNKS)
        nsing = np.bincount(chunk_ur[is_single], minlength=N_CHUNKS)
        per_core.append((ur, inv, first, second, is_single, chunk_ur, npairs, nsing))
    npairs_max = np.max([pc[6] for pc in per_core], axis=0)
    nsing_max = np.max([pc[7] for pc in per_core], axis=0)
    SP = np.where(npairs_max > 0, np.maximum(_roundup(npairs_max, 128), 128), 0)
    SS = np.where(nsing_max > 0, np.maximum(_roundup(nsing_max, 128), 128), 0)
    stg_rows = int((2 * SP + SS).sum())
    return per_core, SP.astype(int), SS.astype(int), stg_rows


def plan_v31(rules_i32, pairing=True, shuffle=False):
    rng = np.random.default_rng(12345)
    shards = rules_i32.reshape(N_CORES, ROWS_PER_CORE).astype(np.int64)

    # greedy group packing (128-row steps): staging rows <= GRP_CAP
    bounds = [0]
    plans = []
    while bounds[-1] < ROWS_PER_CORE:
        a = bounds[-1]
        b = min(a + 34048, ROWS_PER_CORE)
        while True:
            per_core, SP, SS, stg_rows = _plan_group(shards, a, b, pairing)
            if stg_rows <= GRP_CAP or b - a <= 128:
                break
            over = stg_rows - GRP_CAP
            b = a + max(128, (b - a) - _roundup(over, 128))
        assert stg_rows <= GRP_CAP, (a, b, stg_rows)
        bounds.append(b)
        plans.append((a, b, per_core, SP, SS, stg_rows))

    groups_static = []
    g1_parts = [[] for _ in range(N_CORES)]
    g2_parts = [[] for _ in range(N_CORES)]
    for a, b, per_core, SP, SS, stg_rows in plans:
        n = b - a
        assert n % 128 == 0
        slot_base = []
        o = 0
        for k in range(N_CHUNKS):
            slot_base.append((o, o + 2 * SP[k]))  # (pairs base, singles base)
            o += 2 * SP[k] + SS[k]
        Ct = n // 128
        base_cols = Ct // N_SUB
        rem = Ct % N_SUB
        subs = []
        oc = 0
        for s in range(N_SUB):
            cs = base_cols + (1 if s < rem else 0)
            if cs > 0:
                subs.append((oc, cs))
            oc += cs
        groups_static.append(
            dict(a=a, b=b, SP=tuple(SP), SS=tuple(SS), subs=tuple(subs))
        )

        for c in range(N_CORES):
            ur, inv, first, second, is_single, chunk_ur, npairs, nsing = per_core[c]
            pos_ur = np.empty(len(ur), dtype=np.int64)
            for k in range(N_CHUNKS):
                pb, sb = slot_base[k]
                cols_p = SP[k] // 128
                cols_s = SS[k] // 128
                sel = first & (chunk_ur == k)
                rows = ur[sel]
                np_k = len(rows)
                j = rng.permutation(np_k) if shuffle else np.arange(np_k)
                p1 = pb + (j % 128) * (2 * cols_p) + 2 * (j // 128)
                pos_ur[sel] = p1
                sel2 = np.zeros(len(ur), bool)
                sel2[1:] = sel[:-1]
                pos_ur[sel2] = p1 + 1
                if SP[k] > 0:
                    gi = np.zeros(SP[k], dtype=np.int16)
                    gi[j] = (rows - k * CHUNK).astype(np.int16)
                    g1_parts[c].append(_wrap16(gi))
                sel = is_single & (chunk_ur == k)
                rows = ur[sel]
                ns_k = len(rows)
                j = rng.permutation(ns_k) if shuffle else np.arange(ns_k)
                pos_ur[sel] = sb + (j % 128) * cols_s + j // 128
                if SS[k] > 0:
                    gi = np.zeros(SS[k], dtype=np.int16)
                    gi[j] = (rows - k * CHUNK).astype(np.int16)
                    g1_parts[c].append(_wrap16(gi))
            stagpos = pos_ur[inv]
            assert stagpos.max() < GRP_CAP
            for o_s, C_s in subs:
                block = stagpos[128 * o_s : 128 * (o_s + C_s)]
                PM = block.reshape(128, C_s)
                stream = PM.T.ravel().astype(np.int16)
                g2_parts[c].append(_wrap16(stream))

    g1idx_w = np.stack([np.concatenate(p, axis=1) for p in g1_parts])
    g2idx_w = np.stack([np.concatenate(p, axis=1) for p in g2_parts])
    static = tuple(
        (g["a"], g["b"], g["SP"], g["SS"], g["subs"]) for g in groups_static
    )
    return static, g1idx_w, g2idx_w


def build_v31(static, dynamic_reps=False, reps=1, mode="full", qmode="rr", psl=0, order="group"):
    nc = bacc.Bacc("TRN2", target_bir_lowering=False, num_swdge_queues=4)
    tot1 = sum(sum(sp) + sum(ss) for (_a, _b, sp, ss, _subs) in static)
    tot2 = ROWS_PER_CORE
    features = nc.dram_tensor(
        "features", [N_ACTIVE, C], mybir.dt.float32, kind="ExternalInput"
    )
    g1idx = nc.dram_tensor(
        "g1idx", [P, tot1 // 16], mybir.dt.int16, kind="ExternalInput"
    )
    g2idx = nc.dram_tensor(
        "g2idx", [P, tot2 // 16], mybir.dt.int16, kind="ExternalInput"
    )
    if dynamic_reps:
        cnt = nc.dram_tensor("cnt", [1, 16], mybir.dt.int32, kind="ExternalInput")
    out = nc.dram_tensor(
        "out", [ROWS_PER_CORE, C], mybir.dt.float32, kind="ExternalOutput"
    )

    qn = [0]
    qload = [0, 0, 0, 0]

    def next_q(descs=0):
        if qmode == "greedy":
            q = min(range(4), key=lambda i: qload[i])
            qload[q] += descs
            return q
        q = qn[0] % 4
        qn[0] += 1
        return q

    with tile.TileContext(nc) as tc:
        with (
            tc.tile_pool(name="idx", bufs=1) as idx_pool,
            tc.tile_pool(name="p1", bufs=1) as p1_pool,
            tc.tile_pool(name="p2", bufs=1) as p2_pool,
            tc.tile_pool(name="stg", bufs=1, space="DRAM") as stg_pool,
        ):
            g1idx_t = idx_pool.tile([P, tot1 // 16], mybir.dt.int16, tag="g1")
            g2idx_t = idx_pool.tile([P, tot2 // 16], mybir.dt.int16, tag="g2")
            nc.sync.dma_start(out=g1idx_t[:], in_=g1idx[:])
            nc.sync.dma_start(out=g2idx_t[:], in_=g2idx[:])
            if dynamic_reps:
                cnt_t = idx_pool.tile([1, 16], mybir.dt.int32, tag="cnt")
                nc.sync.dma_start(out=cnt_t[:], in_=cnt[:])

            def body():
                col1 = [0]
                col2 = [0]
                stg_tiles = {}

                def do_group(gi_, do_p1, do_p2):
                    (a, b, SP, SS, subs) = static[gi_]
                    if do_p1:
                        stg_rows = sum(2 * s for s in SP) + sum(SS)
                        stg_new = stg_pool.tile(
                            [stg_rows, C], mybir.dt.float32, tag=f"stg{gi_}"
                        )
                        stg_tiles[gi_] = stg_new
                    stg_t = stg_tiles[gi_]
                    slot = 0
                    for k in range(N_CHUNKS):
                        w_base = k * CHUNK
                        n_window = min(CHUNK, N_ACTIVE - w_base - 1)
                        s_p = SP[k]
                        if s_p > 0:
                            if do_p1 and mode != "p2":
                                cols_p = s_p // 128
                                w = s_p // 16
                                base_ap = features[w_base : w_base + 2]
                                pair_ap = AP(
                                    base_ap.tensor,
                                    base_ap.offset,
                                    [[C, n_window], [1, 2 * C]],
                                )
                                t = p1_pool.tile(
                                    [P, cols_p, 2 * C],
                                    mybir.dt.float32,
                                    tag=f"p1p_{k}",
                                )
                                step = psl if psl else s_p
                                for j0 in range(0, s_p, step):
                                    j1 = min(j0 + step, s_p)
                                    nc.gpsimd.dma_gather(
                                        t[:, j0 // 128 : j1 // 128],
                                        pair_ap,
                                        g1idx_t[
                                            :,
                                            col1[0] + j0 // 16 : col1[0] + j1 // 16,
                                        ],
                                        num_idxs=j1 - j0,
                                        num_idxs_reg=j1 - j0,
                                        elem_size=2 * C,
                                        elem_step=C,
                                        single_packet=False,
                                        queue_num=next_q(j1 - j0),
                                    )
                                nc.sync.dma_start(
                                    out=stg_t[slot : slot + 2 * s_p].rearrange(
                                        "(p n) c -> p (n c)", p=P
                                    ),
                                    in_=t[:].rearrange("p n c -> p (n c)"),
                                )
                            col1[0] += s_p // 16
                            slot += 2 * s_p
                        s_s = SS[k]
                        if s_s > 0:
                            if do_p1 and mode != "p2":
                                cols_s = s_s // 128
                                w = s_s // 16
                                c_end = min(w_base + CHUNK, N_ACTIVE)
                                t = p1_pool.tile(
                                    [P, cols_s, C], mybir.dt.float32, tag=f"p1s_{k}"
                                )
                                step = psl if psl else s_s
                                for j0 in range(0, s_s, step):
                                    j1 = min(j0 + step, s_s)
                                    nc.gpsimd.dma_gather(
                                        t[:, j0 // 128 : j1 // 128],
                                        features[w_base:c_end],
                                        g1idx_t[
                                            :,
                                            col1[0] + j0 // 16 : col1[0] + j1 // 16,
                                        ],
                                        num_idxs=j1 - j0,
                                        num_idxs_reg=j1 - j0,
                                        elem_size=C,
                                        elem_step=C,
                                        single_packet=False,
                                        queue_num=next_q(j1 - j0),
                                    )
                                nc.sync.dma_start(
                                    out=stg_t[slot : slot + s_s].rearrange(
                                        "(p n) c -> p (n c)", p=P
                                    ),
                                    in_=t[:].rearrange("p n c -> p (n c)"),
                                )
                            col1[0] += s_s // 16
                            slot += s_s
                    if (not do_p2) or mode == "p1":
                        if not do_p1:
                            pass
                        if do_p2 or mode == "p1":
                            col2[0] += (b - a) // 16
                        return
                    for si_, (o_s, C_s) in enumerate(subs):
                        n_i = 128 * C_s
                        w = n_i // 16
                        t2 = p2_pool.tile(
                            [P, C_s, C], mybir.dt.float32, tag=f"p2_{si_}"
                        )
                        nc.gpsimd.dma_gather(
                            t2[:],
                            stg_t[:],
                            g2idx_t[:, col2[0] : col2[0] + w],
                            num_idxs=n_i,
                            num_idxs_reg=n_i,
                            elem_size=C,
                            elem_step=C,
                            single_packet=False,
                            queue_num=next_q(n_i),
                        )
                        col2[0] += w
                        A_s = a + 128 * o_s
                        nc.sync.dma_start(
                            out=out[A_s : A_s + n_i].rearrange(
                                "(p n) c -> p (n c)", p=P
                            ),
                            in_=t2[:].rearrange("p n c -> p (n c)"),
                        )

                if order == "phase":
                    for gi_ in range(len(static)):
                        do_group(gi_, True, False)
                    for gi_ in range(len(static)):
                        do_group(gi_, False, True)
                else:
                    for gi_ in range(len(static)):
                        do_group(gi_, True, True)

            if dynamic_reps:
                rregs = nc.alloc_registers("reps")
                nc.regs_load(rregs, cnt_t[:1, 15:16])
                reps_val = nc.snap(rregs, donate=True)
                with tc.For_i(0, reps_val) as _i:
                    body()
            else:
                for _ in range(reps):
                    body()
    nc.finalize()
    return nc


def run(features, rules):
    from concourse.bass_utils import run_bass_kernel_spmd

    features = np.ascontiguousarray(np.asarray(features), dtype=np.float32)
    rules_i32 = np.ascontiguousarray(np.asarray(rules)).astype(np.int32)

    static, g1idx_w, g2idx_w = plan_v31(rules_i32, pairing=True, shuffle=True)
    key = ("v32", static)
    if _cache.get("key") != key:
        _cache["nc"] = build_v31(static, qmode="greedy", order="phase")
        _cache["key"] = key
    nc = _cache["nc"]

    in_maps = [
        {"features": features, "g1idx": g1idx_w[c], "g2idx": g2idx_w[c]}
        for c in range(N_CORES)
    ]
    res = run_bass_kernel_spmd(nc, in_maps, list(range(N_CORES)))
    full = np.concatenate([res.results[c]["out"] for c in range(N_CORES)], axis=0)
    return full, res


def kernel(**inputs):
    full, _ = run(inputs["features"], inputs["rules"])
    return full


def measure_hw_ns(features, rules, r_lo=64, r_hi=1088, mode="full"):
    from bench import BassRunner

    features = np.ascontiguousarray(np.asarray(features), dtype=np.float32)
    rules_i32 = np.ascontiguousarray(np.asarray(rules)).astype(np.int32)
    static, g1idx_w, g2idx_w = plan_v31(rules_i32, pairing=True, shuffle=True)
    nc = build_v31(static, dynamic_reps=True, mode=mode, qmode="greedy", order="phase")

    def with_reps(r):
        return [
            {
                "features": features,
                "g1idx": g1idx_w[c],
                "g2idx": g2idx_w[c],
                "cnt": np.array([[0] * 15 + [r]], np.int32),
            }
            for c in range(N_CORES)
        ]

    runner = BassRunner(nc, with_reps(r_lo))
    return runner.time_reps(with_reps, r_lo, r_hi, verbose=True)


# revision 7
# speedup vs baseline: 2.0141x; 1.0826x over previous
"""v3.2: two-phase gather, dedup + pair-merge, phase-ordered emission.

Phase 1 gathers each group's UNIQUE table rows (shuffled stream order);
consecutive-row runs use 512B descriptors (elem_size=128, elem_step=64 via an
overlapping strided AP). Staging writes and final output writes ride HWDGE
with large contiguous descriptors. Phase 2 gathers staging positions in
output order. ALL phase-1 groups are emitted before ANY phase-2 group so the
4 SWDGE rings never stall on a staging barrier; instructions are assigned to
rings by greedy descriptor-count balancing.

Phase 1 gathers each group's UNIQUE table rows once; runs of consecutive
table rows are covered by 512B descriptors (elem_size=128 f32, elem_step=64
-> one descriptor fetches rows r and r+1 via an overlapping strided AP).
Phase 2 gathers staging positions in output order (8 sub-gathers/group) and
writes `out` with big contiguous HWDGE descriptors.
"""

import numpy as np

import concourse.bacc as bacc
import concourse.mybir as mybir
import concourse.tile as tile
from concourse.ap import AP

N_ACTIVE = 200000
C = 64
N_ROWS = 524288
N_CORES = 8
ROWS_PER_CORE = N_ROWS // N_CORES  # 65536
P = 128

CHUNK = 32768
N_CHUNKS = (N_ACTIVE + CHUNK - 1) // CHUNK  # 7
GRP_CAP = 32768  # int16 staging reach per group
N_SUB = 8

_cache = {}


def _wrap16(a):
    w = a.reshape(-1, 16).T
    return np.tile(w, (8, 1)).copy()


def _roundup(x, m):
    return -(-x // m) * m


def _pair_structure(ur):
    """Greedy pairing of sorted unique rows into pairs/singles.

    Returns (first_of_pair_mask, second_of_pair_mask)."""
    m = len(ur)
    if m == 0:
        return np.zeros(0, bool), np.zeros(0, bool)
    new_run = np.ones(m, bool)
    new_run[1:] = np.diff(ur) != 1
    run_id = np.cumsum(new_run) - 1
    run_start = np.flatnonzero(new_run)
    run_len = np.diff(np.append(run_start, m))
    pos = np.arange(m) - run_start[run_id]
    first = (pos % 2 == 0) & (pos + 1 < run_len[run_id])
    second = np.zeros(m, bool)
    second[1:] = first[:-1]
    return first, second


def _plan_group(shards, a, b, pairing=True):
    """Per-core pair/single bucket structure for rows [a:b)."""
    per_core = []
    for c in range(N_CORES):
        idx = shards[c, a:b]
        ur, inv = np.unique(idx, return_inverse=True)
        if pairing:
            first, second = _pair_structure(ur)
        else:
            first = np.zeros(len(ur), bool)
            second = np.zeros(len(ur), bool)
        chunk_ur = (ur >> 15).astype(np.int64)
        is_single = ~(first | second)
        npairs = np.bincount(chunk_ur[first], minlength=N_CHUNKS)
        nsing = np.bincount(chunk_ur[is_single], minlength=N_CHUNKS)
        per_core.append((ur, inv, first, second, is_single, chunk_ur, npairs, nsing))
    npairs_max = np.max([pc[6] for pc in per_core], axis=0)
    nsing_max = np.max([pc[7] for pc in per_core], axis=0)
    SP = np.where(npairs_max > 0, np.maximum(_roundup(npairs_max, 128), 128), 0)
    SS = np.where(nsing_max > 0, np.maximum(_roundup(nsing_max, 128), 128), 0)
    stg_rows = int((2 * SP + SS).sum())
    return per_core, SP.astype(int), SS.astype(int), stg_rows


def plan_v31(rules_i32, pairing=True, shuffle=False):
    rng = np.random.default_rng(12345)
    shards = rules_i32.reshape(N_CORES, ROWS_PER_CORE).astype(np.int64)

    # greedy group packing (128-row steps): staging rows <= GRP_CAP
    bounds = [0]
    plans = []
    while bounds[-1] < ROWS_PER_CORE:
        a = bounds[-1]
        b = min(a + 34048, ROWS_PER_CORE)
        while True:
            per_core, SP, SS, stg_rows = _plan_group(shards, a, b, pairing)
            if stg_rows <= GRP_CAP or b - a <= 128:
                break
            over = stg_rows - GRP_CAP
            b = a + max(128, (b - a) - _roundup(over, 128))
        assert stg_rows <= GRP_CAP, (a, b, stg_rows)
        bounds.append(b)
        plans.append((a, b, per_core, SP, SS, stg_rows))

    groups_static = []
    g1_parts = [[] for _ in range(N_CORES)]
    g2_parts = [[] for _ in range(N_CORES)]
    for a, b, per_core, SP, SS, stg_rows in plans:
        n = b - a
        assert n % 128 == 0
        slot_base = []
        o = 0
        for k in range(N_CHUNKS):
            slot_base.append((o, o + 2 * SP[k]))  # (pairs base, singles base)
            o += 2 * SP[k] + SS[k]
        Ct = n // 128
        base_cols = Ct // N_SUB
        rem = Ct % N_SUB
        subs = []
        oc = 0
        for s in range(N_SUB):
            cs = base_cols + (1 if s < rem else 0)
            if cs > 0:
                subs.append((oc, cs))
            oc += cs
        groups_static.append(
            dict(a=a, b=b, SP=tuple(SP), SS=tuple(SS), subs=tuple(subs))
        )

        for c in range(N_CORES):
            ur, inv, first, second, is_single, chunk_ur, npairs, nsing = per_core[c]
            pos_ur = np.empty(len(ur), dtype=np.int64)
            for k in range(N_CHUNKS):
                pb, sb = slot_base[k]
                cols_p = SP[k] // 128
                cols_s = SS[k] // 128
                sel = first & (chunk_ur == k)
                rows = ur[sel]
                np_k = len(rows)
                j = rng.permutation(np_k) if shuffle else np.arange(np_k)
                p1 = pb + (j % 128) * (2 * cols_p) + 2 * (j // 128)
                pos_ur[sel] = p1
                sel2 = np.zeros(len(ur), bool)
                sel2[1:] = sel[:-1]
                pos_ur[sel2] = p1 + 1
                if SP[k] > 0:
                    gi = np.zeros(SP[k], dtype=np.int16)
                    gi[j] = (rows - k * CHUNK).astype(np.int16)
                    g1_parts[c].append(_wrap16(gi))
                sel = is_single & (chunk_ur == k)
                rows = ur[sel]
                ns_k = len(rows)
                j = rng.permutation(ns_k) if shuffle else np.arange(ns_k)
                pos_ur[sel] = sb + (j % 128) * cols_s + j // 128
                if SS[k] > 0:
                    gi = np.zeros(SS[k], dtype=np.int16)
                    gi[j] = (rows - k * CHUNK).astype(np.int16)
                    g1_parts[c].append(_wrap16(gi))
            stagpos = pos_ur[inv]
            assert stagpos.max() < GRP_CAP
            for o_s, C_s in subs:
                block = stagpos[128 * o_s : 128 * (o_s + C_s)]
                PM = block.reshape(128, C_s)
                stream = PM.T.ravel().astype(np.int16)
                g2_parts[c].append(_wrap16(stream))

    g1idx_w = np.stack([np.concatenate(p, axis=1) for p in g1_parts])
    g2idx_w = np.stack([np.concatenate(p, axis=1) for p in g2_parts])
    static = tuple(
        (g["a"], g["b"], g["SP"], g["SS"], g["subs"]) for g in groups_static
    )
    return static, g1idx_w, g2idx_w


def build_v31(static, dynamic_reps=False, reps=1, mode="full", qmode="rr", psl=0, order="group", stg_bufs=1, weng=False):
    nc = bacc.Bacc("TRN2", target_bir_lowering=False, num_swdge_queues=4)
    tot1 = sum(sum(sp) + sum(ss) for (_a, _b, sp, ss, _subs) in static)
    tot2 = ROWS_PER_CORE
    features = nc.dram_tensor(
        "features", [N_ACTIVE, C], mybir.dt.float32, kind="ExternalInput"
    )
    g1idx = nc.dram_tensor(
        "g1idx", [P, tot1 // 16], mybir.dt.int16, kind="ExternalInput"
    )
    g2idx = nc.dram_tensor(
        "g2idx", [P, tot2 // 16], mybir.dt.int16, kind="ExternalInput"
    )
    if dynamic_reps:
        cnt = nc.dram_tensor("cnt", [1, 16], mybir.dt.int32, kind="ExternalInput")
    out = nc.dram_tensor(
        "out", [ROWS_PER_CORE, C], mybir.dt.float32, kind="ExternalOutput"
    )

    qn = [0]
    qload = [0, 0, 0, 0]

    def next_q(descs=0):
        if qmode == "greedy":
            q = min(range(4), key=lambda i: qload[i])
            qload[q] += descs
            return q
        q = qn[0] % 4
        qn[0] += 1
        return q

    wn = [0]

    def wengine():
        if not weng:
            return nc.sync
        engs = [nc.sync, nc.scalar]
        e = engs[wn[0] % len(engs)]
        wn[0] += 1
        return e

    with tile.TileContext(nc) as tc:
        with (
            tc.tile_pool(name="idx", bufs=1) as idx_pool,
            tc.tile_pool(name="p1", bufs=1) as p1_pool,
            tc.tile_pool(name="p2", bufs=1) as p2_pool,
            tc.tile_pool(name="stg", bufs=stg_bufs, space="DRAM") as stg_pool,
        ):
            g1idx_t = idx_pool.tile([P, tot1 // 16], mybir.dt.int16, tag="g1")
            g2idx_t = idx_pool.tile([P, tot2 // 16], mybir.dt.int16, tag="g2")
            nc.sync.dma_start(out=g1idx_t[:], in_=g1idx[:])
            nc.sync.dma_start(out=g2idx_t[:], in_=g2idx[:])
            if dynamic_reps:
                cnt_t = idx_pool.tile([1, 16], mybir.dt.int32, tag="cnt")
                nc.sync.dma_start(out=cnt_t[:], in_=cnt[:])

            def body():
                col1 = [0]
                col2 = [0]
                stg_tiles = {}

                def do_group(gi_, do_p1, do_p2):
                    (a, b, SP, SS, subs) = static[gi_]
                    if do_p1:
                        stg_rows = sum(2 * s for s in SP) + sum(SS)
                        stg_new = stg_pool.tile(
                            [stg_rows, C], mybir.dt.float32, tag=f"stg{gi_}"
                        )
                        stg_tiles[gi_] = stg_new
                    stg_t = stg_tiles[gi_]
                    slot = 0
                    for k in range(N_CHUNKS):
                        w_base = k * CHUNK
                        n_window = min(CHUNK, N_ACTIVE - w_base - 1)
                        s_p = SP[k]
                        if s_p > 0:
                            if do_p1 and mode != "p2":
                                cols_p = s_p // 128
                                w = s_p // 16
                                base_ap = features[w_base : w_base + 2]
                                pair_ap = AP(
                                    base_ap.tensor,
                                    base_ap.offset,
                                    [[C, n_window], [1, 2 * C]],
                                )
                                t = p1_pool.tile(
                                    [P, cols_p, 2 * C],
                                    mybir.dt.float32,
                                    tag=f"p1p_{k}",
                                )
                                step = psl if psl else s_p
                                for j0 in range(0, s_p, step):
                                    j1 = min(j0 + step, s_p)
                                    nc.gpsimd.dma_gather(
                                        t[:, j0 // 128 : j1 // 128],
                                        pair_ap,
                                        g1idx_t[
                                            :,
                                            col1[0] + j0 // 16 : col1[0] + j1 // 16,
                                        ],
                                        num_idxs=j1 - j0,
                                        num_idxs_reg=j1 - j0,
                                        elem_size=2 * C,
                                        elem_step=C,
                                        single_packet=False,
                                        queue_num=next_q(j1 - j0),
                                    )
                                wengine().dma_start(
                                    out=stg_t[slot : slot + 2 * s_p].rearrange(
                                        "(p n) c -> p (n c)", p=P
                                    ),
                                    in_=t[:].rearrange("p n c -> p (n c)"),
                                )
                            col1[0] += s_p // 16
                            slot += 2 * s_p
                        s_s = SS[k]
                        if s_s > 0:
                            if do_p1 and mode != "p2":
                                cols_s = s_s // 128
                                w = s_s // 16
                                c_end = min(w_base + CHUNK, N_ACTIVE)
                                t = p1_pool.tile(
                                    [P, cols_s, C], mybir.dt.float32, tag=f"p1s_{k}"
                                )
                                step = psl if psl else s_s
                                for j0 in range(0, s_s, step):
                                    j1 = min(j0 + step, s_s)
                                    nc.gpsimd.dma_gather(
                                        t[:, j0 // 128 : j1 // 128],
                                        features[w_base:c_end],
                                        g1idx_t[
                                            :,
                                            col1[0] + j0 // 16 : col1[0] + j1 // 16,
                                        ],
                                        num_idxs=j1 - j0,
                                        num_idxs_reg=j1 - j0,
                                        elem_size=C,
                                        elem_step=C,
                                        single_packet=False,
                                        queue_num=next_q(j1 - j0),
                                    )
                                wengine().dma_start(
                                    out=stg_t[slot : slot + s_s].rearrange(
                                        "(p n) c -> p (n c)", p=P
                                    ),
                                    in_=t[:].rearrange("p n c -> p (n c)"),
                                )
                            col1[0] += s_s // 16
                            slot += s_s
                    if (not do_p2) or mode == "p1":
                        if not do_p1:
                            pass
                        if do_p2 or mode == "p1":
                            col2[0] += (b - a) // 16
                        return
                    for si_, (o_s, C_s) in enumerate(subs):
                        n_i = 128 * C_s
                        w = n_i // 16
                        t2 = p2_pool.tile(
                            [P, C_s, C], mybir.dt.float32, tag=f"p2_{si_}"
                        )
                        nc.gpsimd.dma_gather(
                            t2[:],
                            stg_t[:],
                            g2idx_t[:, col2[0] : col2[0] + w],
                            num_idxs=n_i,
                            num_idxs_reg=n_i,
                            elem_size=C,
                            elem_step=C,
                            single_packet=False,
                            queue_num=next_q(n_i),
                        )
                        col2[0] += w
                        A_s = a + 128 * o_s
                        wengine().dma_start(
                            out=out[A_s : A_s + n_i].rearrange(
                                "(p n) c -> p (n c)", p=P
                            ),
                            in_=t2[:].rearrange("p n c -> p (n c)"),
                        )

                if order == "phase":
                    for gi_ in range(len(static)):
                        do_group(gi_, True, False)
                    for gi_ in range(len(static)):
                        do_group(gi_, False, True)
                else:
                    for gi_ in range(len(static)):
                        do_group(gi_, True, True)

            if dynamic_reps:
                rregs = nc.alloc_registers("reps")
                nc.regs_load(rregs, cnt_t[:1, 15:16])
                reps_val = nc.snap(rregs, donate=True)
                with tc.For_i(0, reps_val) as _i:
                    body()
            else:
                for _ in range(reps):
                    body()
    nc.finalize()
    return nc


def run(features, rules):
    from concourse.bass_utils import run_bass_kernel_spmd

    features = np.ascontiguousarray(np.asarray(features), dtype=np.float32)
    rules_i32 = np.ascontiguousarray(np.asarray(rules)).astype(np.int32)

    static, g1idx_w, g2idx_w = plan_v31(rules_i32, pairing=True, shuffle=True)
    key = ("v32", static)
    if _cache.get("key") != key:
        _cache["nc"] = build_v31(static, qmode="greedy", order="phase")
        _cache["key"] = key
    nc = _cache["nc"]

    in_maps = [
        {"features": features, "g1idx": g1idx_w[c], "g2idx": g2idx_w[c]}
        for c in range(N_CORES)
    ]
    res = run_bass_kernel_spmd(nc, in_maps, list(range(N_CORES)))
    full = np.concatenate([res.results[c]["out"] for c in range(N_CORES)], axis=0)
    return full, res


def kernel(**inputs):
    full, _ = run(inputs["features"], inputs["rules"])
    return full


def measure_hw_ns(features, rules, r_lo=64, r_hi=1088, mode="full"):
    from bench import BassRunner

    features = np.ascontiguousarray(np.asarray(features), dtype=np.float32)
    rules_i32 = np.ascontiguousarray(np.asarray(rules)).astype(np.int32)
    static, g1idx_w, g2idx_w = plan_v31(rules_i32, pairing=True, shuffle=True)
    nc = build_v31(static, dynamic_reps=True, mode=mode, qmode="greedy", order="phase")

    def with_reps(r):
        return [
            {
                "features": features,
                "g1idx": g1idx_w[c],
                "g2idx": g2idx_w[c],
                "cnt": np.array([[0] * 15 + [r]], np.int32),
            }
            for c in range(N_CORES)
        ]

    runner = BassRunner(nc, with_reps(r_lo))
    return runner.time_reps(with_reps, r_lo, r_hi, verbose=True)


# revision 8
# speedup vs baseline: 2.0927x; 1.0390x over previous
"""v3.3: two-phase gather, dedup + ALIGNED pair-merge (512B stride-512B
descriptors, the fast SWDGE path), phase-ordered emission.

Phase 1 gathers each group's UNIQUE table rows (shuffled stream order);
consecutive-row runs use 512B descriptors (elem_size=128, elem_step=64 via an
overlapping strided AP). Staging writes and final output writes ride HWDGE
with large contiguous descriptors. Phase 2 gathers staging positions in
output order. ALL phase-1 groups are emitted before ANY phase-2 group so the
4 SWDGE rings never stall on a staging barrier; instructions are assigned to
rings by greedy descriptor-count balancing.

Phase 1 gathers each group's UNIQUE table rows once; runs of consecutive
table rows are covered by 512B descriptors (elem_size=128 f32, elem_step=64
-> one descriptor fetches rows r and r+1 via an overlapping strided AP).
Phase 2 gathers staging positions in output order (8 sub-gathers/group) and
writes `out` with big contiguous HWDGE descriptors.
"""

import numpy as np

import concourse.bacc as bacc
import concourse.mybir as mybir
import concourse.tile as tile
from concourse.ap import AP

N_ACTIVE = 200000
C = 64
N_ROWS = 524288
N_CORES = 8
ROWS_PER_CORE = N_ROWS // N_CORES  # 65536
P = 128

CHUNK = 32768
N_CHUNKS = (N_ACTIVE + CHUNK - 1) // CHUNK  # 7
GRP_CAP = 32768  # int16 staging reach per group
N_SUB = 8

_cache = {}


def _wrap16(a):
    w = a.reshape(-1, 16).T
    return np.tile(w, (8, 1)).copy()


def _roundup(x, m):
    return -(-x // m) * m


def _pair_structure(ur):
    """Greedy pairing of sorted unique rows into pairs/singles.

    Returns (first_of_pair_mask, second_of_pair_mask)."""
    m = len(ur)
    if m == 0:
        return np.zeros(0, bool), np.zeros(0, bool)
    nxt = np.zeros(m, bool)
    nxt[:-1] = np.diff(ur) == 1
    first = (ur % 2 == 0) & nxt  # aligned: rows (2m, 2m+1)
    second = np.zeros(m, bool)
    second[1:] = first[:-1]
    return first, second


def _plan_group(shards, a, b, pairing=True):
    """Per-core pair/single bucket structure for rows [a:b)."""
    per_core = []
    for c in range(N_CORES):
        idx = shards[c, a:b]
        ur, inv = np.unique(idx, return_inverse=True)
        if pairing:
            first, second = _pair_structure(ur)
        else:
            first = np.zeros(len(ur), bool)
            second = np.zeros(len(ur), bool)
        chunk_ur = (ur >> 15).astype(np.int64)
        pchunk_ur = (ur >> 16).astype(np.int64)  # pair idx (ur>>1) >> 15
        is_single = ~(first | second)
        npairs = np.bincount(pchunk_ur[first], minlength=N_CHUNKS)
        nsing = np.bincount(chunk_ur[is_single], minlength=N_CHUNKS)
        per_core.append(
            (ur, inv, first, second, is_single, chunk_ur, npairs, nsing, pchunk_ur)
        )
    npairs_max = np.max([pc[6] for pc in per_core], axis=0)
    nsing_max = np.max([pc[7] for pc in per_core], axis=0)
    SP = np.where(npairs_max > 0, np.maximum(_roundup(npairs_max, 128), 128), 0)
    SS = np.where(nsing_max > 0, np.maximum(_roundup(nsing_max, 128), 128), 0)
    stg_rows = int((2 * SP + SS).sum())
    return per_core, SP.astype(int), SS.astype(int), stg_rows


def plan_v31(rules_i32, pairing=True, shuffle=False):
    rng = np.random.default_rng(12345)
    shards = rules_i32.reshape(N_CORES, ROWS_PER_CORE).astype(np.int64)

    # greedy group packing (128-row steps): staging rows <= GRP_CAP
    bounds = [0]
    plans = []
    while bounds[-1] < ROWS_PER_CORE:
        a = bounds[-1]
        b = min(a + 34048, ROWS_PER_CORE)
        while True:
            per_core, SP, SS, stg_rows = _plan_group(shards, a, b, pairing)
            if stg_rows <= GRP_CAP or b - a <= 128:
                break
            over = stg_rows - GRP_CAP
            b = a + max(128, (b - a) - _roundup(over, 128))
        assert stg_rows <= GRP_CAP, (a, b, stg_rows)
        bounds.append(b)
        plans.append((a, b, per_core, SP, SS, stg_rows))

    groups_static = []
    g1_parts = [[] for _ in range(N_CORES)]
    g2_parts = [[] for _ in range(N_CORES)]
    for a, b, per_core, SP, SS, stg_rows in plans:
        n = b - a
        assert n % 128 == 0
        slot_base = []
        o = 0
        for k in range(N_CHUNKS):
            slot_base.append((o, o + 2 * SP[k]))  # (pairs base, singles base)
            o += 2 * SP[k] + SS[k]
        Ct = n // 128
        base_cols = Ct // N_SUB
        rem = Ct % N_SUB
        subs = []
        oc = 0
        for s in range(N_SUB):
            cs = base_cols + (1 if s < rem else 0)
            if cs > 0:
                subs.append((oc, cs))
            oc += cs
        groups_static.append(
            dict(a=a, b=b, SP=tuple(SP), SS=tuple(SS), subs=tuple(subs))
        )

        for c in range(N_CORES):
            (ur, inv, first, second, is_single, chunk_ur, npairs, nsing,
             pchunk_ur) = per_core[c]
            pos_ur = np.empty(len(ur), dtype=np.int64)
            for k in range(N_CHUNKS):
                pb, sb = slot_base[k]
                cols_p = SP[k] // 128
                cols_s = SS[k] // 128
                sel = first & (pchunk_ur == k)
                pm = ur[sel] >> 1  # aligned pair index
                np_k = len(pm)
                j = rng.permutation(np_k) if shuffle else np.arange(np_k)
                p1 = pb + (j % 128) * (2 * cols_p) + 2 * (j // 128)
                pos_ur[sel] = p1
                sel2 = np.zeros(len(ur), bool)
                sel2[1:] = sel[:-1]
                pos_ur[sel2] = p1 + 1
                if SP[k] > 0:
                    gi = np.zeros(SP[k], dtype=np.int16)
                    gi[j] = (pm - k * CHUNK).astype(np.int16)
                    g1_parts[c].append(_wrap16(gi))
                sel = is_single & (chunk_ur == k)
                rows = ur[sel]
                ns_k = len(rows)
                j = rng.permutation(ns_k) if shuffle else np.arange(ns_k)
                pos_ur[sel] = sb + (j % 128) * cols_s + j // 128
                if SS[k] > 0:
                    gi = np.zeros(SS[k], dtype=np.int16)
                    gi[j] = (rows - k * CHUNK).astype(np.int16)
                    g1_parts[c].append(_wrap16(gi))
            stagpos = pos_ur[inv]
            assert stagpos.max() < GRP_CAP
            for o_s, C_s in subs:
                block = stagpos[128 * o_s : 128 * (o_s + C_s)]
                PM = block.reshape(128, C_s)
                stream = PM.T.ravel().astype(np.int16)
                g2_parts[c].append(_wrap16(stream))

    g1idx_w = np.stack([np.concatenate(p, axis=1) for p in g1_parts])
    g2idx_w = np.stack([np.concatenate(p, axis=1) for p in g2_parts])
    static = tuple(
        (g["a"], g["b"], g["SP"], g["SS"], g["subs"]) for g in groups_static
    )
    return static, g1idx_w, g2idx_w


def build_v31(static, dynamic_reps=False, reps=1, mode="full", qmode="rr", psl=0, order="group", stg_bufs=1, weng=False):
    nc = bacc.Bacc("TRN2", target_bir_lowering=False, num_swdge_queues=4)
    tot1 = sum(sum(sp) + sum(ss) for (_a, _b, sp, ss, _subs) in static)
    tot2 = ROWS_PER_CORE
    features = nc.dram_tensor(
        "features", [N_ACTIVE, C], mybir.dt.float32, kind="ExternalInput"
    )
    g1idx = nc.dram_tensor(
        "g1idx", [P, tot1 // 16], mybir.dt.int16, kind="ExternalInput"
    )
    g2idx = nc.dram_tensor(
        "g2idx", [P, tot2 // 16], mybir.dt.int16, kind="ExternalInput"
    )
    if dynamic_reps:
        cnt = nc.dram_tensor("cnt", [1, 16], mybir.dt.int32, kind="ExternalInput")
    out = nc.dram_tensor(
        "out", [ROWS_PER_CORE, C], mybir.dt.float32, kind="ExternalOutput"
    )

    qn = [0]
    qload = [0, 0, 0, 0]

    def next_q(descs=0):
        if qmode == "greedy":
            q = min(range(4), key=lambda i: qload[i])
            qload[q] += descs
            return q
        q = qn[0] % 4
        qn[0] += 1
        return q

    wn = [0]

    def wengine():
        if not weng:
            return nc.sync
        engs = [nc.sync, nc.scalar]
        e = engs[wn[0] % len(engs)]
        wn[0] += 1
        return e

    with tile.TileContext(nc) as tc:
        with (
            tc.tile_pool(name="idx", bufs=1) as idx_pool,
            tc.tile_pool(name="p1", bufs=1) as p1_pool,
            tc.tile_pool(name="p2", bufs=1) as p2_pool,
            tc.tile_pool(name="stg", bufs=stg_bufs, space="DRAM") as stg_pool,
        ):
            g1idx_t = idx_pool.tile([P, tot1 // 16], mybir.dt.int16, tag="g1")
            g2idx_t = idx_pool.tile([P, tot2 // 16], mybir.dt.int16, tag="g2")
            nc.sync.dma_start(out=g1idx_t[:], in_=g1idx[:])
            nc.sync.dma_start(out=g2idx_t[:], in_=g2idx[:])
            if dynamic_reps:
                cnt_t = idx_pool.tile([1, 16], mybir.dt.int32, tag="cnt")
                nc.sync.dma_start(out=cnt_t[:], in_=cnt[:])

            def body():
                col1 = [0]
                col2 = [0]
                stg_tiles = {}

                def do_group(gi_, do_p1, do_p2):
                    (a, b, SP, SS, subs) = static[gi_]
                    if do_p1:
                        stg_rows = sum(2 * s for s in SP) + sum(SS)
                        stg_new = stg_pool.tile(
                            [stg_rows, C], mybir.dt.float32, tag=f"stg{gi_}"
                        )
                        stg_tiles[gi_] = stg_new
                    stg_t = stg_tiles[gi_]
                    slot = 0
                    for k in range(N_CHUNKS):
                        w_base = k * CHUNK
                        pm_base = k * CHUNK
                        n_window = max(1, min(CHUNK, N_ACTIVE // 2 - pm_base))
                        s_p = SP[k]
                        if s_p > 0:
                            if do_p1 and mode != "p2":
                                cols_p = s_p // 128
                                w = s_p // 16
                                base_ap = features[2 * pm_base : 2 * pm_base + 2]
                                pair_ap = AP(
                                    base_ap.tensor,
                                    base_ap.offset,
                                    [[2 * C, n_window], [1, 2 * C]],
                                )
                                t = p1_pool.tile(
                                    [P, cols_p, 2 * C],
                                    mybir.dt.float32,
                                    tag=f"p1p_{k}",
                                )
                                step = psl if psl else s_p
                                for j0 in range(0, s_p, step):
                                    j1 = min(j0 + step, s_p)
                                    nc.gpsimd.dma_gather(
                                        t[:, j0 // 128 : j1 // 128],
                                        pair_ap,
                                        g1idx_t[
                                            :,
                                            col1[0] + j0 // 16 : col1[0] + j1 // 16,
                                        ],
                                        num_idxs=j1 - j0,
                                        num_idxs_reg=j1 - j0,
                                        elem_size=2 * C,
                                        elem_step=2 * C,
                                        single_packet=False,
                                        queue_num=next_q(j1 - j0),
                                    )
                                wengine().dma_start(
                                    out=stg_t[slot : slot + 2 * s_p].rearrange(
                                        "(p n) c -> p (n c)", p=P
                                    ),
                                    in_=t[:].rearrange("p n c -> p (n c)"),
                                )
                            col1[0] += s_p // 16
                            slot += 2 * s_p
                        s_s = SS[k]
                        if s_s > 0:
                            if do_p1 and mode != "p2":
                                cols_s = s_s // 128
                                w = s_s // 16
                                c_end = min(w_base + CHUNK, N_ACTIVE)
                                t = p1_pool.tile(
                                    [P, cols_s, C], mybir.dt.float32, tag=f"p1s_{k}"
                                )
                                step = psl if psl else s_s
                                for j0 in range(0, s_s, step):
                                    j1 = min(j0 + step, s_s)
                                    nc.gpsimd.dma_gather(
                                        t[:, j0 // 128 : j1 // 128],
                                        features[w_base:c_end],
                                        g1idx_t[
                                            :,
                                            col1[0] + j0 // 16 : col1[0] + j1 // 16,
                                        ],
                                        num_idxs=j1 - j0,
                                        num_idxs_reg=j1 - j0,
                                        elem_size=C,
                                        elem_step=C,
                                        single_packet=False,
                                        queue_num=next_q(j1 - j0),
                                    )
                                wengine().dma_start(
                                    out=stg_t[slot : slot + s_s].rearrange(
                                        "(p n) c -> p (n c)", p=P
                                    ),
                                    in_=t[:].rearrange("p n c -> p (n c)"),
                                )
                            col1[0] += s_s // 16
                            slot += s_s
                    if (not do_p2) or mode == "p1":
                        if not do_p1:
                            pass
                        if do_p2 or mode == "p1":
                            col2[0] += (b - a) // 16
                        return
                    for si_, (o_s, C_s) in enumerate(subs):
                        n_i = 128 * C_s
                        w = n_i // 16
                        t2 = p2_pool.tile(
                            [P, C_s, C], mybir.dt.float32, tag=f"p2_{si_}"
                        )
                        nc.gpsimd.dma_gather(
                            t2[:],
                            stg_t[:],
                            g2idx_t[:, col2[0] : col2[0] + w],
                            num_idxs=n_i,
                            num_idxs_reg=n_i,
                            elem_size=C,
                            elem_step=C,
                            single_packet=False,
                            queue_num=next_q(n_i),
                        )
                        col2[0] += w
                        A_s = a + 128 * o_s
                        wengine().dma_start(
                            out=out[A_s : A_s + n_i].rearrange(
                                "(p n) c -> p (n c)", p=P
                            ),
                            in_=t2[:].rearrange("p n c -> p (n c)"),
                        )

                if order == "phase":
                    for gi_ in range(len(static)):
                        do_group(gi_, True, False)
                    for gi_ in range(len(static)):
                        do_group(gi_, False, True)
                else:
                    for gi_ in range(len(static)):
                        do_group(gi_, True, True)

            if dynamic_reps:
                rregs = nc.alloc_registers("reps")
                nc.regs_load(rregs, cnt_t[:1, 15:16])
                reps_val = nc.snap(rregs, donate=True)
                with tc.For_i(0, reps_val) as _i:
                    body()
            else:
                for _ in range(reps):
                    body()
    nc.finalize()
    return nc


def run(features, rules):
    from concourse.bass_utils import run_bass_kernel_spmd

    features = np.ascontiguousarray(np.asarray(features), dtype=np.float32)
    rules_i32 = np.ascontiguousarray(np.asarray(rules)).astype(np.int32)

    static, g1idx_w, g2idx_w = plan_v31(rules_i32, pairing=True, shuffle=True)
    key = ("v32", static)
    if _cache.get("key") != key:
        _cache["nc"] = build_v31(static, qmode="greedy", order="phase")
        _cache["key"] = key
    nc = _cache["nc"]

    in_maps = [
        {"features": features, "g1idx": g1idx_w[c], "g2idx": g2idx_w[c]}
        for c in range(N_CORES)
    ]
    res = run_bass_kernel_spmd(nc, in_maps, list(range(N_CORES)))
    full = np.concatenate([res.results[c]["out"] for c in range(N_CORES)], axis=0)
    return full, res


def kernel(**inputs):
    full, _ = run(inputs["features"], inputs["rules"])
    return full


def measure_hw_ns(features, rules, r_lo=64, r_hi=1088, mode="full"):
    from bench import BassRunner

    features = np.ascontiguousarray(np.asarray(features), dtype=np.float32)
    rules_i32 = np.ascontiguousarray(np.asarray(rules)).astype(np.int32)
    static, g1idx_w, g2idx_w = plan_v31(rules_i32, pairing=True, shuffle=True)
    nc = build_v31(static, dynamic_reps=True, mode=mode, qmode="greedy", order="phase")

    def with_reps(r):
        return [
            {
                "features": features,
                "g1idx": g1idx_w[c],
                "g2idx": g2idx_w[c],
                "cnt": np.array([[0] * 15 + [r]], np.int32),
            }
            for c in range(N_CORES)
        ]

    runner = BassRunner(nc, with_reps(r_lo))
    return runner.time_reps(with_reps, r_lo, r_hi, verbose=True)


# revision 9
# speedup vs baseline: 2.1482x; 1.0266x over previous
"""v3.3: two-phase gather, dedup + ALIGNED pair-merge, phase-ordered emission.

Phase 1 gathers each group's UNIQUE table rows (shuffled stream order).
Aligned row pairs (2m, 2m+1) both present are fetched by one 512B
descriptor with elem_step == elem_size (the fast non-overlapping SWDGE
path; overlapping stride-256B descriptors cost ~3x more). Staging writes
and final output writes ride HWDGE with large contiguous descriptors.
Phase 2 gathers staging positions in output order. ALL phase-1 groups are
emitted before ANY phase-2 group so the 4 SWDGE rings never stall on a
staging barrier; instructions go to rings by greedy descriptor-count
balancing.
"""

import numpy as np

import concourse.bacc as bacc
import concourse.mybir as mybir
import concourse.tile as tile
from concourse.ap import AP

N_ACTIVE = 200000
C = 64
N_ROWS = 524288
N_CORES = 8
ROWS_PER_CORE = N_ROWS // N_CORES  # 65536
P = 128

CHUNK = 32768
N_CHUNKS = (N_ACTIVE + CHUNK - 1) // CHUNK  # 7
GRP_CAP = 32768  # int16 staging reach per group
N_SUB = 8

_cache = {}


def _wrap16(a):
    w = a.reshape(-1, 16).T
    return np.tile(w, (8, 1)).copy()


def _roundup(x, m):
    return -(-x // m) * m


def _pair_structure(ur):
    """Greedy pairing of sorted unique rows into pairs/singles.

    Returns (first_of_pair_mask, second_of_pair_mask)."""
    m = len(ur)
    if m == 0:
        return np.zeros(0, bool), np.zeros(0, bool)
    nxt = np.zeros(m, bool)
    nxt[:-1] = np.diff(ur) == 1
    first = (ur % 2 == 0) & nxt  # aligned: rows (2m, 2m+1)
    second = np.zeros(m, bool)
    second[1:] = first[:-1]
    return first, second


def _plan_group(shards, a, b, pairing=True):
    """Per-core pair/single bucket structure for rows [a:b)."""
    per_core = []
    for c in range(N_CORES):
        idx = shards[c, a:b]
        ur, inv = np.unique(idx, return_inverse=True)
        if pairing:
            first, second = _pair_structure(ur)
        else:
            first = np.zeros(len(ur), bool)
            second = np.zeros(len(ur), bool)
        chunk_ur = (ur >> 15).astype(np.int64)
        pchunk_ur = (ur >> 16).astype(np.int64)  # pair idx (ur>>1) >> 15
        is_single = ~(first | second)
        npairs = np.bincount(pchunk_ur[first], minlength=N_CHUNKS)
        nsing = np.bincount(chunk_ur[is_single], minlength=N_CHUNKS)
        per_core.append(
            (ur, inv, first, second, is_single, chunk_ur, npairs, nsing, pchunk_ur)
        )
    npairs_max = np.max([pc[6] for pc in per_core], axis=0)
    nsing_max = np.max([pc[7] for pc in per_core], axis=0)
    SP = np.where(npairs_max > 0, np.maximum(_roundup(npairs_max, 128), 128), 0)
    SS = np.where(nsing_max > 0, np.maximum(_roundup(nsing_max, 128), 128), 0)
    stg_rows = int((2 * SP + SS).sum())
    return per_core, SP.astype(int), SS.astype(int), stg_rows


def plan_v31(rules_i32, pairing=True, shuffle=False):
    rng = np.random.default_rng(12345)
    shards = rules_i32.reshape(N_CORES, ROWS_PER_CORE).astype(np.int64)

    # greedy group packing (128-row steps): staging rows <= GRP_CAP
    bounds = [0]
    plans = []
    while bounds[-1] < ROWS_PER_CORE:
        a = bounds[-1]
        b = min(a + 34048, ROWS_PER_CORE)
        while True:
            per_core, SP, SS, stg_rows = _plan_group(shards, a, b, pairing)
            if stg_rows <= GRP_CAP or b - a <= 128:
                break
            over = stg_rows - GRP_CAP
            b = a + max(128, (b - a) - _roundup(over, 128))
        assert stg_rows <= GRP_CAP, (a, b, stg_rows)
        bounds.append(b)
        plans.append((a, b, per_core, SP, SS, stg_rows))

    groups_static = []
    g1_parts = [[] for _ in range(N_CORES)]
    g2_parts = [[] for _ in range(N_CORES)]
    for a, b, per_core, SP, SS, stg_rows in plans:
        n = b - a
        assert n % 128 == 0
        slot_base = []
        o = 0
        for k in range(N_CHUNKS):
            slot_base.append((o, o + 2 * SP[k]))  # (pairs base, singles base)
            o += 2 * SP[k] + SS[k]
        Ct = n // 128
        base_cols = Ct // N_SUB
        rem = Ct % N_SUB
        subs = []
        oc = 0
        for s in range(N_SUB):
            cs = base_cols + (1 if s < rem else 0)
            if cs > 0:
                subs.append((oc, cs))
            oc += cs
        groups_static.append(
            dict(a=a, b=b, SP=tuple(SP), SS=tuple(SS), subs=tuple(subs))
        )

        for c in range(N_CORES):
            (ur, inv, first, second, is_single, chunk_ur, npairs, nsing,
             pchunk_ur) = per_core[c]
            pos_ur = np.empty(len(ur), dtype=np.int64)
            for k in range(N_CHUNKS):
                pb, sb = slot_base[k]
                cols_p = SP[k] // 128
                cols_s = SS[k] // 128
                sel = first & (pchunk_ur == k)
                pm = ur[sel] >> 1  # aligned pair index
                np_k = len(pm)
                j = rng.permutation(np_k) if shuffle else np.arange(np_k)
                p1 = pb + (j % 128) * (2 * cols_p) + 2 * (j // 128)
                pos_ur[sel] = p1
                sel2 = np.zeros(len(ur), bool)
                sel2[1:] = sel[:-1]
                pos_ur[sel2] = p1 + 1
                if SP[k] > 0:
                    gi = np.zeros(SP[k], dtype=np.int16)
                    gi[j] = (pm - k * CHUNK).astype(np.int16)
                    g1_parts[c].append(_wrap16(gi))
                sel = is_single & (chunk_ur == k)
                rows = ur[sel]
                ns_k = len(rows)
                j = rng.permutation(ns_k) if shuffle else np.arange(ns_k)
                pos_ur[sel] = sb + (j % 128) * cols_s + j // 128
                if SS[k] > 0:
                    gi = np.zeros(SS[k], dtype=np.int16)
                    gi[j] = (rows - k * CHUNK).astype(np.int16)
                    g1_parts[c].append(_wrap16(gi))
            stagpos = pos_ur[inv]
            assert stagpos.max() < GRP_CAP
            for o_s, C_s in subs:
                block = stagpos[128 * o_s : 128 * (o_s + C_s)]
                PM = block.reshape(128, C_s)
                stream = PM.T.ravel().astype(np.int16)
                g2_parts[c].append(_wrap16(stream))

    g1idx_w = np.stack([np.concatenate(p, axis=1) for p in g1_parts])
    g2idx_w = np.stack([np.concatenate(p, axis=1) for p in g2_parts])
    static = tuple(
        (g["a"], g["b"], g["SP"], g["SS"], g["subs"]) for g in groups_static
    )
    return static, g1idx_w, g2idx_w


def build_v31(static, dynamic_reps=False, reps=1, mode="full", qmode="rr", psl=0, order="group", stg_bufs=1, weng=False):
    nc = bacc.Bacc("TRN2", target_bir_lowering=False, num_swdge_queues=4)
    tot1 = sum(sum(sp) + sum(ss) for (_a, _b, sp, ss, _subs) in static)
    tot2 = ROWS_PER_CORE
    features = nc.dram_tensor(
        "features", [N_ACTIVE, C], mybir.dt.float32, kind="ExternalInput"
    )
    g1idx = nc.dram_tensor(
        "g1idx", [P, tot1 // 16], mybir.dt.int16, kind="ExternalInput"
    )
    g2idx = nc.dram_tensor(
        "g2idx", [P, tot2 // 16], mybir.dt.int16, kind="ExternalInput"
    )
    if dynamic_reps:
        cnt = nc.dram_tensor("cnt", [1, 16], mybir.dt.int32, kind="ExternalInput")
    out = nc.dram_tensor(
        "out", [ROWS_PER_CORE, C], mybir.dt.float32, kind="ExternalOutput"
    )

    qn = [0]
    qload = [0, 0, 0, 0]

    def next_q(descs=0):
        if qmode == "greedy":
            q = min(range(4), key=lambda i: qload[i])
            qload[q] += descs
            return q
        q = qn[0] % 4
        qn[0] += 1
        return q

    wn = [0]

    def wengine():
        if not weng:
            return nc.sync
        engs = [nc.sync, nc.scalar]
        e = engs[wn[0] % len(engs)]
        wn[0] += 1
        return e

    with tile.TileContext(nc) as tc:
        with (
            tc.tile_pool(name="idx", bufs=1) as idx_pool,
            tc.tile_pool(name="p1", bufs=1) as p1_pool,
            tc.tile_pool(name="p2", bufs=1) as p2_pool,
            tc.tile_pool(name="stg", bufs=stg_bufs, space="DRAM") as stg_pool,
        ):
            g1idx_t = idx_pool.tile([P, tot1 // 16], mybir.dt.int16, tag="g1")
            g2idx_t = idx_pool.tile([P, tot2 // 16], mybir.dt.int16, tag="g2")
            nc.sync.dma_start(out=g1idx_t[:], in_=g1idx[:])
            nc.sync.dma_start(out=g2idx_t[:], in_=g2idx[:])
            if dynamic_reps:
                cnt_t = idx_pool.tile([1, 16], mybir.dt.int32, tag="cnt")
                nc.sync.dma_start(out=cnt_t[:], in_=cnt[:])

            def body():
                col1 = [0]
                col2 = [0]
                stg_tiles = {}

                def do_group(gi_, do_p1, do_p2):
                    (a, b, SP, SS, subs) = static[gi_]
                    if do_p1:
                        stg_rows = sum(2 * s for s in SP) + sum(SS)
                        stg_new = stg_pool.tile(
                            [stg_rows, C], mybir.dt.float32, tag=f"stg{gi_}"
                        )
                        stg_tiles[gi_] = stg_new
                    stg_t = stg_tiles[gi_]
                    slot = 0
                    for k in range(N_CHUNKS):
                        w_base = k * CHUNK
                        pm_base = k * CHUNK
                        n_window = max(1, min(CHUNK, N_ACTIVE // 2 - pm_base))
                        s_p = SP[k]
                        if s_p > 0:
                            if do_p1 and mode != "p2":
                                cols_p = s_p // 128
                                w = s_p // 16
                                base_ap = features[2 * pm_base : 2 * pm_base + 2]
                                pair_ap = AP(
                                    base_ap.tensor,
                                    base_ap.offset,
                                    [[2 * C, n_window], [1, 2 * C]],
                                )
                                t = p1_pool.tile(
                                    [P, cols_p, 2 * C],
                                    mybir.dt.float32,
                                    tag=f"p1p_{k}",
                                )
                                step = psl if psl else s_p
                                for j0 in range(0, s_p, step):
                                    j1 = min(j0 + step, s_p)
                                    nc.gpsimd.dma_gather(
                                        t[:, j0 // 128 : j1 // 128],
                                        pair_ap,
                                        g1idx_t[
                                            :,
                                            col1[0] + j0 // 16 : col1[0] + j1 // 16,
                                        ],
                                        num_idxs=j1 - j0,
                                        num_idxs_reg=j1 - j0,
                                        elem_size=2 * C,
                                        elem_step=2 * C,
                                        single_packet=False,
                                        queue_num=next_q(j1 - j0),
                                    )
                                wengine().dma_start(
                                    out=stg_t[slot : slot + 2 * s_p].rearrange(
                                        "(p n) c -> p (n c)", p=P
                                    ),
                                    in_=t[:].rearrange("p n c -> p (n c)"),
                                )
                            col1[0] += s_p // 16
                            slot += 2 * s_p
                        s_s = SS[k]
                        if s_s > 0:
                            if do_p1 and mode != "p2":
                                cols_s = s_s // 128
                                w = s_s // 16
                                c_end = min(w_base + CHUNK, N_ACTIVE)
                                t = p1_pool.tile(
                                    [P, cols_s, C], mybir.dt.float32, tag=f"p1s_{k}"
                                )
                                step = psl if psl else s_s
                                for j0 in range(0, s_s, step):
                                    j1 = min(j0 + step, s_s)
                                    nc.gpsimd.dma_gather(
                                        t[:, j0 // 128 : j1 // 128],
                                        features[w_base:c_end],
                                        g1idx_t[
                                            :,
                                            col1[0] + j0 // 16 : col1[0] + j1 // 16,
                                        ],
                                        num_idxs=j1 - j0,
                                        num_idxs_reg=j1 - j0,
                                        elem_size=C,
                                        elem_step=C,
                                        single_packet=False,
                                        queue_num=next_q(j1 - j0),
                                    )
                                wengine().dma_start(
                                    out=stg_t[slot : slot + s_s].rearrange(
                                        "(p n) c -> p (n c)", p=P
                                    ),
                                    in_=t[:].rearrange("p n c -> p (n c)"),
                                )
                            col1[0] += s_s // 16
                            slot += s_s
                    if (not do_p2) or mode == "p1":
                        if not do_p1:
                            pass
                        if do_p2 or mode == "p1":
                            col2[0] += (b - a) // 16
                        return
                    for si_, (o_s, C_s) in enumerate(subs):
                        n_i = 128 * C_s
                        w = n_i // 16
                        t2 = p2_pool.tile(
                            [P, C_s, C], mybir.dt.float32, tag=f"p2_{si_}"
                        )
                        nc.gpsimd.dma_gather(
                            t2[:],
                            stg_t[:],
                            g2idx_t[:, col2[0] : col2[0] + w],
                            num_idxs=n_i,
                            num_idxs_reg=n_i,
                            elem_size=C,
                            elem_step=C,
                            single_packet=False,
                            queue_num=next_q(n_i),
                        )
                        col2[0] += w
                        A_s = a + 128 * o_s
                        wengine().dma_start(
                            out=out[A_s : A_s + n_i].rearrange(
                                "(p n) c -> p (n c)", p=P
                            ),
                            in_=t2[:].rearrange("p n c -> p (n c)"),
                        )

                if order == "phase":
                    for gi_ in range(len(static)):
                        do_group(gi_, True, False)
                    for gi_ in range(len(static)):
                        do_group(gi_, False, True)
                else:
                    for gi_ in range(len(static)):
                        do_group(gi_, True, True)

            if dynamic_reps:
                rregs = nc.alloc_registers("reps")
                nc.regs_load(rregs, cnt_t[:1, 15:16])
                reps_val = nc.snap(rregs, donate=True)
                with tc.For_i(0, reps_val) as _i:
                    body()
            else:
                for _ in range(reps):
                    body()
    nc.finalize()
    return nc


def run(features, rules):
    from concourse.bass_utils import run_bass_kernel_spmd

    features = np.ascontiguousarray(np.asarray(features), dtype=np.float32)
    rules_i32 = np.ascontiguousarray(np.asarray(rules)).astype(np.int32)

    static, g1idx_w, g2idx_w = plan_v31(rules_i32, pairing=True, shuffle=True)
    key = ("v32", static)
    if _cache.get("key") != key:
        _cache["nc"] = build_v31(static, qmode="greedy", order="phase")
        _cache["key"] = key
    nc = _cache["nc"]

    in_maps = [
        {"features": features, "g1idx": g1idx_w[c], "g2idx": g2idx_w[c]}
        for c in range(N_CORES)
    ]
    res = run_bass_kernel_spmd(nc, in_maps, list(range(N_CORES)))
    full = np.concatenate([res.results[c]["out"] for c in range(N_CORES)], axis=0)
    return full, res


def kernel(**inputs):
    full, _ = run(inputs["features"], inputs["rules"])
    return full


def measure_hw_ns(features, rules, r_lo=64, r_hi=1088, mode="full"):
    from bench import BassRunner

    features = np.ascontiguousarray(np.asarray(features), dtype=np.float32)
    rules_i32 = np.ascontiguousarray(np.asarray(rules)).astype(np.int32)
    static, g1idx_w, g2idx_w = plan_v31(rules_i32, pairing=True, shuffle=True)
    nc = build_v31(static, dynamic_reps=True, mode=mode, qmode="greedy", order="phase")

    def with_reps(r):
        return [
            {
                "features": features,
                "g1idx": g1idx_w[c],
                "g2idx": g2idx_w[c],
                "cnt": np.array([[0] * 15 + [r]], np.int32),
            }
            for c in range(N_CORES)
        ]

    runner = BassRunner(nc, with_reps(r_lo))
    return runner.time_reps(with_reps, r_lo, r_hi, verbose=True)


# revision 10
# speedup vs baseline: 2.1567x; 1.0040x over previous
"""v3.5 (16-pad num_idxs, 128-pad rectangles): two-phase gather, dedup + ALIGNED pair-merge (512B stride-512B
descriptors, the fast SWDGE path), phase-ordered emission.

Phase 1 gathers each group's UNIQUE table rows (shuffled stream order);
consecutive-row runs use 512B descriptors (elem_size=128, elem_step=64 via an
overlapping strided AP). Staging writes and final output writes ride HWDGE
with large contiguous descriptors. Phase 2 gathers staging positions in
output order. ALL phase-1 groups are emitted before ANY phase-2 group so the
4 SWDGE rings never stall on a staging barrier; instructions are assigned to
rings by greedy descriptor-count balancing.

Phase 1 gathers each group's UNIQUE table rows once; runs of consecutive
table rows are covered by 512B descriptors (elem_size=128 f32, elem_step=64
-> one descriptor fetches rows r and r+1 via an overlapping strided AP).
Phase 2 gathers staging positions in output order (8 sub-gathers/group) and
writes `out` with big contiguous HWDGE descriptors.
"""

import numpy as np

import concourse.bacc as bacc
import concourse.mybir as mybir
import concourse.tile as tile
from concourse.ap import AP

N_ACTIVE = 200000
C = 64
N_ROWS = 524288
N_CORES = 8
ROWS_PER_CORE = N_ROWS // N_CORES  # 65536
P = 128

CHUNK = 32768
N_CHUNKS = (N_ACTIVE + CHUNK - 1) // CHUNK  # 7
GRP_CAP = 32768  # int16 staging reach per group
N_SUB = 8

_cache = {}


def _wrap16(a):
    w = a.reshape(-1, 16).T
    return np.tile(w, (8, 1)).copy()


def _roundup(x, m):
    return -(-x // m) * m


def _pair_structure(ur):
    """Greedy pairing of sorted unique rows into pairs/singles.

    Returns (first_of_pair_mask, second_of_pair_mask)."""
    m = len(ur)
    if m == 0:
        return np.zeros(0, bool), np.zeros(0, bool)
    nxt = np.zeros(m, bool)
    nxt[:-1] = np.diff(ur) == 1
    first = (ur % 2 == 0) & nxt  # aligned: rows (2m, 2m+1)
    second = np.zeros(m, bool)
    second[1:] = first[:-1]
    return first, second


def _plan_group(shards, a, b, pairing=True):
    """Per-core pair/single bucket structure for rows [a:b)."""
    per_core = []
    for c in range(N_CORES):
        idx = shards[c, a:b]
        ur, inv = np.unique(idx, return_inverse=True)
        if pairing:
            first, second = _pair_structure(ur)
        else:
            first = np.zeros(len(ur), bool)
            second = np.zeros(len(ur), bool)
        chunk_ur = (ur >> 15).astype(np.int64)
        pchunk_ur = (ur >> 16).astype(np.int64)  # pair idx (ur>>1) >> 15
        is_single = ~(first | second)
        npairs = np.bincount(pchunk_ur[first], minlength=N_CHUNKS)
        nsing = np.bincount(chunk_ur[is_single], minlength=N_CHUNKS)
        per_core.append(
            (ur, inv, first, second, is_single, chunk_ur, npairs, nsing, pchunk_ur)
        )
    npairs_max = np.max([pc[6] for pc in per_core], axis=0)
    nsing_max = np.max([pc[7] for pc in per_core], axis=0)
    SP = np.where(npairs_max > 0, np.maximum(_roundup(npairs_max, 16), 16), 0)
    SS = np.where(nsing_max > 0, np.maximum(_roundup(nsing_max, 16), 16), 0)
    stg_rows = int((2 * _roundup(SP, 128) + _roundup(SS, 128)).sum())
    return per_core, SP.astype(int), SS.astype(int), stg_rows


def plan_v31(rules_i32, pairing=True, shuffle=False):
    rng = np.random.default_rng(12345)
    shards = rules_i32.reshape(N_CORES, ROWS_PER_CORE).astype(np.int64)

    # greedy group packing (128-row steps): staging rows <= GRP_CAP
    bounds = [0]
    plans = []
    while bounds[-1] < ROWS_PER_CORE:
        a = bounds[-1]
        b = min(a + 34048, ROWS_PER_CORE)
        while True:
            per_core, SP, SS, stg_rows = _plan_group(shards, a, b, pairing)
            if stg_rows <= GRP_CAP or b - a <= 128:
                break
            over = stg_rows - GRP_CAP
            b = a + max(128, (b - a) - _roundup(over, 128))
        assert stg_rows <= GRP_CAP, (a, b, stg_rows)
        bounds.append(b)
        plans.append((a, b, per_core, SP, SS, stg_rows))

    groups_static = []
    g1_parts = [[] for _ in range(N_CORES)]
    g2_parts = [[] for _ in range(N_CORES)]
    for a, b, per_core, SP, SS, stg_rows in plans:
        n = b - a
        assert n % 128 == 0
        slot_base = []
        o = 0
        for k in range(N_CHUNKS):
            rp, rs = _roundup(SP[k], 128), _roundup(SS[k], 128)
            slot_base.append((o, o + 2 * rp))  # (pairs base, singles base)
            o += 2 * rp + rs
        Ct = n // 128
        base_cols = Ct // N_SUB
        rem = Ct % N_SUB
        subs = []
        oc = 0
        for s in range(N_SUB):
            cs = base_cols + (1 if s < rem else 0)
            if cs > 0:
                subs.append((oc, cs))
            oc += cs
        groups_static.append(
            dict(a=a, b=b, SP=tuple(SP), SS=tuple(SS), subs=tuple(subs))
        )

        for c in range(N_CORES):
            (ur, inv, first, second, is_single, chunk_ur, npairs, nsing,
             pchunk_ur) = per_core[c]
            pos_ur = np.empty(len(ur), dtype=np.int64)
            for k in range(N_CHUNKS):
                pb, sb = slot_base[k]
                cols_p = _roundup(SP[k], 128) // 128
                cols_s = _roundup(SS[k], 128) // 128
                sel = first & (pchunk_ur == k)
                pm = ur[sel] >> 1  # aligned pair index
                np_k = len(pm)
                j = rng.permutation(np_k) if shuffle else np.arange(np_k)
                p1 = pb + (j % 128) * (2 * cols_p) + 2 * (j // 128)
                pos_ur[sel] = p1
                sel2 = np.zeros(len(ur), bool)
                sel2[1:] = sel[:-1]
                pos_ur[sel2] = p1 + 1
                if SP[k] > 0:
                    gi = np.zeros(SP[k], dtype=np.int16)
                    gi[j] = (pm - k * CHUNK).astype(np.int16)
                    g1_parts[c].append(_wrap16(gi))
                sel = is_single & (chunk_ur == k)
                rows = ur[sel]
                ns_k = len(rows)
                j = rng.permutation(ns_k) if shuffle else np.arange(ns_k)
                pos_ur[sel] = sb + (j % 128) * cols_s + j // 128
                if SS[k] > 0:
                    gi = np.zeros(SS[k], dtype=np.int16)
                    gi[j] = (rows - k * CHUNK).astype(np.int16)
                    g1_parts[c].append(_wrap16(gi))
            stagpos = pos_ur[inv]
            assert stagpos.max() < GRP_CAP
            for o_s, C_s in subs:
                block = stagpos[128 * o_s : 128 * (o_s + C_s)]
                PM = block.reshape(128, C_s)
                stream = PM.T.ravel().astype(np.int16)
                g2_parts[c].append(_wrap16(stream))

    g1idx_w = np.stack([np.concatenate(p, axis=1) for p in g1_parts])
    g2idx_w = np.stack([np.concatenate(p, axis=1) for p in g2_parts])
    static = tuple(
        (g["a"], g["b"], g["SP"], g["SS"], g["subs"]) for g in groups_static
    )
    return static, g1idx_w, g2idx_w


def build_v31(static, dynamic_reps=False, reps=1, mode="full", qmode="rr", psl=0, order="group", stg_bufs=1, weng=False, p1_bufs=1, p2_fold=False):
    nc = bacc.Bacc("TRN2", target_bir_lowering=False, num_swdge_queues=4)
    tot1 = sum(sum(sp) + sum(ss) for (_a, _b, sp, ss, _subs) in static)
    tot2 = ROWS_PER_CORE
    features = nc.dram_tensor(
        "features", [N_ACTIVE, C], mybir.dt.float32, kind="ExternalInput"
    )
    g1idx = nc.dram_tensor(
        "g1idx", [P, tot1 // 16], mybir.dt.int16, kind="ExternalInput"
    )
    g2idx = nc.dram_tensor(
        "g2idx", [P, tot2 // 16], mybir.dt.int16, kind="ExternalInput"
    )
    if dynamic_reps:
        cnt = nc.dram_tensor("cnt", [1, 16], mybir.dt.int32, kind="ExternalInput")
    out = nc.dram_tensor(
        "out", [ROWS_PER_CORE, C], mybir.dt.float32, kind="ExternalOutput"
    )

    qn = [0]
    qload = [0, 0, 0, 0]

    def next_q(descs=0):
        if qmode == "greedy":
            q = min(range(4), key=lambda i: qload[i])
            qload[q] += descs
            return q
        q = qn[0] % 4
        qn[0] += 1
        return q

    wn = [0]

    def wengine():
        if not weng:
            return nc.sync
        engs = [nc.sync, nc.scalar]
        e = engs[wn[0] % len(engs)]
        wn[0] += 1
        return e

    with tile.TileContext(nc) as tc:
        with (
            tc.tile_pool(name="idx", bufs=1) as idx_pool,
            tc.tile_pool(name="p1", bufs=p1_bufs) as p1_pool,
            tc.tile_pool(name="p2", bufs=1) as p2_pool,
            tc.tile_pool(name="stg", bufs=stg_bufs, space="DRAM") as stg_pool,
        ):
            g1idx_t = idx_pool.tile([P, tot1 // 16], mybir.dt.int16, tag="g1")
            g2idx_t = idx_pool.tile([P, tot2 // 16], mybir.dt.int16, tag="g2")
            nc.sync.dma_start(out=g1idx_t[:], in_=g1idx[:])
            nc.sync.dma_start(out=g2idx_t[:], in_=g2idx[:])
            if dynamic_reps:
                cnt_t = idx_pool.tile([1, 16], mybir.dt.int32, tag="cnt")
                nc.sync.dma_start(out=cnt_t[:], in_=cnt[:])

            def body():
                col1 = [0]
                col2 = [0]
                stg_tiles = {}

                def do_group(gi_, do_p1, do_p2):
                    (a, b, SP, SS, subs) = static[gi_]
                    if do_p1:
                        stg_rows = sum(
                            2 * _roundup(s, 128) for s in SP
                        ) + sum(_roundup(s, 128) for s in SS)
                        stg_new = stg_pool.tile(
                            [stg_rows, C], mybir.dt.float32, tag=f"stg{gi_}"
                        )
                        stg_tiles[gi_] = stg_new
                    stg_t = stg_tiles[gi_]
                    slot = 0
                    for k in range(N_CHUNKS):
                        w_base = k * CHUNK
                        pm_base = k * CHUNK
                        n_window = max(1, min(CHUNK, N_ACTIVE // 2 - pm_base))
                        s_p = SP[k]
                        rect_p = _roundup(s_p, 128)
                        if s_p > 0:
                            if do_p1 and mode != "p2":
                                cols_p = rect_p // 128
                                w = s_p // 16
                                base_ap = features[2 * pm_base : 2 * pm_base + 2]
                                pair_ap = AP(
                                    base_ap.tensor,
                                    base_ap.offset,
                                    [[2 * C, n_window], [1, 2 * C]],
                                )
                                t = p1_pool.tile(
                                    [P, cols_p, 2 * C],
                                    mybir.dt.float32,
                                    tag=f"p1p_{k}",
                                )
                                step = psl if psl else s_p
                                for j0 in range(0, s_p, step):
                                    j1 = min(j0 + step, s_p)
                                    nc.gpsimd.dma_gather(
                                        t[:, j0 // 128 : -(-j1 // 128)],
                                        pair_ap,
                                        g1idx_t[
                                            :,
                                            col1[0] + j0 // 16 : col1[0] + j1 // 16,
                                        ],
                                        num_idxs=j1 - j0,
                                        num_idxs_reg=j1 - j0,
                                        elem_size=2 * C,
                                        elem_step=2 * C,
                                        single_packet=False,
                                        queue_num=next_q(j1 - j0),
                                    )
                                wengine().dma_start(
                                    out=stg_t[slot : slot + 2 * rect_p].rearrange(
                                        "(p n) c -> p (n c)", p=P
                                    ),
                                    in_=t[:].rearrange("p n c -> p (n c)"),
                                )
                            col1[0] += s_p // 16
                            slot += 2 * rect_p
                        s_s = SS[k]
                        rect_s = _roundup(s_s, 128)
                        if s_s > 0:
                            if do_p1 and mode != "p2":
                                cols_s = rect_s // 128
                                w = s_s // 16
                                c_end = min(w_base + CHUNK, N_ACTIVE)
                                t = p1_pool.tile(
                                    [P, cols_s, C], mybir.dt.float32, tag=f"p1s_{k}"
                                )
                                step = psl if psl else s_s
                                for j0 in range(0, s_s, step):
                                    j1 = min(j0 + step, s_s)
                                    nc.gpsimd.dma_gather(
                                        t[:, j0 // 128 : -(-j1 // 128)],
                                        features[w_base:c_end],
                                        g1idx_t[
                                            :,
                                            col1[0] + j0 // 16 : col1[0] + j1 // 16,
                                        ],
                                        num_idxs=j1 - j0,
                                        num_idxs_reg=j1 - j0,
                                        elem_size=C,
                                        elem_step=C,
                                        single_packet=False,
                                        queue_num=next_q(j1 - j0),
                                    )
                                wengine().dma_start(
                                    out=stg_t[slot : slot + rect_s].rearrange(
                                        "(p n) c -> p (n c)", p=P
                                    ),
                                    in_=t[:].rearrange("p n c -> p (n c)"),
                                )
                            col1[0] += s_s // 16
                            slot += rect_s
                    if (not do_p2) or mode == "p1":
                        if not do_p1:
                            pass
                        if do_p2 or mode == "p1":
                            col2[0] += (b - a) // 16
                        return
                    for si_, (o_s, C_s) in enumerate(subs):
                        n_i = 128 * C_s
                        w = n_i // 16
                        t2 = p2_pool.tile(
                            [P, C_s, C], mybir.dt.float32,
                            tag=f"p2_{si_ % 4 if p2_fold else si_}"
                        )
                        nc.gpsimd.dma_gather(
                            t2[:],
                            stg_t[:],
                            g2idx_t[:, col2[0] : col2[0] + w],
                            num_idxs=n_i,
                            num_idxs_reg=n_i,
                            elem_size=C,
                            elem_step=C,
                            single_packet=False,
                            queue_num=next_q(n_i),
                        )
                        col2[0] += w
                        A_s = a + 128 * o_s
                        wengine().dma_start(
                            out=out[A_s : A_s + n_i].rearrange(
                                "(p n) c -> p (n c)", p=P
                            ),
                            in_=t2[:].rearrange("p n c -> p (n c)"),
                        )

                if order == "phase":
                    for gi_ in range(len(static)):
                        do_group(gi_, True, False)
                    for gi_ in range(len(static)):
                        do_group(gi_, False, True)
                else:
                    for gi_ in range(len(static)):
                        do_group(gi_, True, True)

            if dynamic_reps:
                rregs = nc.alloc_registers("reps")
                nc.regs_load(rregs, cnt_t[:1, 15:16])
                reps_val = nc.snap(rregs, donate=True)
                with tc.For_i(0, reps_val) as _i:
                    body()
            else:
                for _ in range(reps):
                    body()
    nc.finalize()
    return nc


def run(features, rules):
    from concourse.bass_utils import run_bass_kernel_spmd

    features = np.ascontiguousarray(np.asarray(features), dtype=np.float32)
    rules_i32 = np.ascontiguousarray(np.asarray(rules)).astype(np.int32)

    static, g1idx_w, g2idx_w = plan_v31(rules_i32, pairing=True, shuffle=True)
    key = ("v32", static)
    if _cache.get("key") != key:
        _cache["nc"] = build_v31(static, qmode="greedy", order="phase")
        _cache["key"] = key
    nc = _cache["nc"]

    in_maps = [
        {"features": features, "g1idx": g1idx_w[c], "g2idx": g2idx_w[c]}
        for c in range(N_CORES)
    ]
    res = run_bass_kernel_spmd(nc, in_maps, list(range(N_CORES)))
    full = np.concatenate([res.results[c]["out"] for c in range(N_CORES)], axis=0)
    return full, res


def kernel(**inputs):
    full, _ = run(inputs["features"], inputs["rules"])
    return full


def measure_hw_ns(features, rules, r_lo=64, r_hi=1088, mode="full"):
    from bench import BassRunner

    features = np.ascontiguousarray(np.asarray(features), dtype=np.float32)
    rules_i32 = np.ascontiguousarray(np.asarray(rules)).astype(np.int32)
    static, g1idx_w, g2idx_w = plan_v31(rules_i32, pairing=True, shuffle=True)
    nc = build_v31(static, dynamic_reps=True, mode=mode, qmode="greedy", order="phase")

    def with_reps(r):
        return [
            {
                "features": features,
                "g1idx": g1idx_w[c],
                "g2idx": g2idx_w[c],
                "cnt": np.array([[0] * 15 + [r]], np.int32),
            }
            for c in range(N_CORES)
        ]

    runner = BassRunner(nc, with_reps(r_lo))
    return runner.time_reps(with_reps, r_lo, r_hi, verbose=True)
